# revision 27
# baseline (speedup 1.0000x reference)
"""Multi-head attention (B=4, L=1024, D=1024, H=16, DH=64) on 8 TRN2 NeuronCores.

Sharding: data-parallel over batch (4) x tensor-parallel over heads (2).
Core c = 2*b + t computes, for batch b, heads [t*8, (t+1)*8):
    QT = Wq_t^T X^T, KT = Wk_t^T X^T, V = Y Wv_t        (all bf16 matmuls)
    per head: S^T = K_h Q_h^T; P^T = exp(S^T/8);
              [ctx^T; rowsum] = Vaug_h^T P^T;  ctxn = ctx / rowsum
    O_partial = ctxn^T Wo_t                              (f32, two dt-halves)
Host pre-transposes X/Y, casts to bf16, and sums the four f32 partials
(2 tensor-parallel cores x 2 dt-halves) per batch.

Schedule notes (engines execute their streams in order, so emission order is
the software pipeline):
  - Input DMAs are spread across the 3 DMA-capable queues (SP/gpsimd/
    Activation) in priority classes: the sub-prefix that gates the first
    score steps (wq/wk dt0 cols, xt ic0, yt ic0) leads every queue.
  - ~35 dummy matmuls run during the unavoidable input-DMA wait so the PE
    HAM clock-gate warms up (K=8/8) before real matmuls start.
  - Score (ST) phases run as four (ic, jt-range) quarter-passes of
    single-bank [128,512] tiles; each step issues one MM per head with the
    heads' K blocks at partitions 0:64 / 64:128, so both MMs stream
    concurrently in different PE row groups (~216ns per step for both) and
    the next step's weight load pulls ahead into the row group the running
    MM doesn't occupy.
  - The exp drain on the scalar engine is slower than the ST matmuls, so
    fill chains (V projection, next d-tile QT/KT, earlier heads' ctx,
    out-projection partials) are interleaved between steps to keep the
    tensor engine busy.
  - out_a (Wo over dt 0..2) drains during the last score phase; out_c (dt3)
    runs at the tail with drains alternating scalar/vector, output DMAs
    split over two queues, and identity-matmul partition moves instead of
    SBUF->SBUF DMA round-trips for the last normalizes.
"""

import numpy as np
import ml_dtypes

import concourse.tile as tile
import concourse.mybir as mybir
from concourse import bacc
from concourse.bass_utils import run_bass_kernel_spmd
from concourse.masks import make_identity

B, L, D, U, H = 4, 1024, 1024, 1024, 16
DH = U // H          # 64 head dim
TP = 2               # tensor-parallel ways (heads)
DL = U // TP         # 512 local units
HL = H // TP         # 8 local heads
P = 128              # partitions
NI = 512             # matmul free-dim chunk (one PSUM bank of f32)
CC = D // P          # 8 contraction chunks for projections
DT = DL // P         # 4 local d-tiles
IT = L // P          # 8 i/j tiles
NIC = L // NI        # 2 free chunks of 512
N_CORES = 8
N_DUMMY = 35         # HAM warm-up matmuls during the input-DMA wait

BF16 = mybir.dt.bfloat16
F32 = mybir.dt.float32


def _build_kernel():
    nc = bacc.Bacc(
        "TRN2", target_bir_lowering=False, debug=False, num_devices=N_CORES
    )
    xt = nc.dram_tensor("xt", [D, L], BF16, kind="ExternalInput").ap()
    yt = nc.dram_tensor("yt", [D, L], BF16, kind="ExternalInput").ap()
    wq = nc.dram_tensor("wq", [D, DL], BF16, kind="ExternalInput").ap()
    wk = nc.dram_tensor("wk", [D, DL], BF16, kind="ExternalInput").ap()
    wv = nc.dram_tensor("wv", [D, DL], BF16, kind="ExternalInput").ap()
    wo = nc.dram_tensor("wo", [DL, U], BF16, kind="ExternalInput").ap()
    out_a = nc.dram_tensor("out_a", [L, U], BF16, kind="ExternalOutput").ap()
    out_c = nc.dram_tensor("out_c", [L, U], BF16, kind="ExternalOutput").ap()

    with tile.TileContext(nc) as tc:
        _mha_body(tc, out_a, out_c, xt, yt, wq, wk, wv, wo)

    nc.compile()
    return nc


def _mha_body(tc, out_a, out_c, xt, yt, wq, wk, wv, wo):
    nc = tc.nc
    from contextlib import ExitStack

    with ExitStack() as ctx:
        persist = ctx.enter_context(tc.tile_pool(name="persist", bufs=1))
        pt_pool = ctx.enter_context(tc.tile_pool(name="pt", bufs=4))
        # ST half tiles: [P, 512] f32 = 1 bank each
        ps_half = ctx.enter_context(tc.tile_pool(name="ps_half", bufs=4, space="PSUM"))
        # single-bank accumulators (projections, V, ctx, out-proj)
        ps_acc = ctx.enter_context(tc.tile_pool(name="ps_acc", bufs=4, space="PSUM"))
        small = ctx.enter_context(tc.tile_pool(name="small", bufs=4))

        # persistent SBUF tensors
        xt_sb = persist.tile([P, CC, L], BF16, tag="xt")
        yt_sb = persist.tile([P, CC, L], BF16, tag="yt")
        wq_sb = persist.tile([P, CC, DL], BF16, tag="wq")
        wk_sb = persist.tile([P, CC, DL], BF16, tag="wk")
        wv_sb = persist.tile([P, CC, DL], BF16, tag="wv")
        wo_sb = persist.tile([P, DT, U], BF16, tag="wo")
        qt_sb = persist.tile([P, DT, L], BF16, tag="qt")
        kt_sb = persist.tile([P, DT, L], BF16, tag="kt")
        # Vaug: per j-chunk, per head a 128-col block; even h: [V_h | ones],
        # odd h: [ones | V_h] (ctx^T lands on the head's own cx partitions)
        va_sb = persist.tile([P, IT, HL * P], BF16, tag="va")
        cx_sb = persist.tile([P, DT, L], BF16, tag="cx")
        dummy_sb = persist.tile([P, NI], BF16, tag="dummy")

        wq_r = wq.rearrange("(cc p) d -> p cc d", p=P)
        wk_r = wk.rearrange("(cc p) d -> p cc d", p=P)
        wv_r = wv.rearrange("(cc p) d -> p cc d", p=P)
        xt_r = xt.rearrange("(cc p) i -> p cc i", p=P)
        yt_r = yt.rearrange("(cc p) i -> p cc i", p=P)
        wo_r = wo.rearrange("(dt p) o -> p dt o", p=P)

        # ---- input DMAs over 3 hw queues, priority classes per queue ----
        # class1a: wq/wk dt0 cols, xt ic0, yt ic0  (first score quarter)
        # class1b: xt ic1, yt ic1                  (remaining quarters)
        # class2:  wv                              (V fill chains)
        # class3:  wq/wk rest, wo                  (later d-tiles, out-proj)
        nc.vector.memset(dummy_sb[:], 1.0)
        # sync
        nc.sync.dma_start(out=wq_sb[:, :, 0:P], in_=wq_r[:, :, 0:P])
        for cc in range(5):
            nc.sync.dma_start(out=xt_sb[:, cc, 0:NI], in_=xt_r[:, cc, 0:NI])
        for cc in range(5):
            nc.sync.dma_start(out=xt_sb[:, cc, NI:L], in_=xt_r[:, cc, NI:L])
        for cc in range(3):
            nc.sync.dma_start(out=wv_sb[:, cc], in_=wv_r[:, cc])
        nc.sync.dma_start(out=wq_sb[:, :, P:DL], in_=wq_r[:, :, P:DL])
        # gpsimd
        nc.gpsimd.dma_start(out=wk_sb[:, :, 0:P], in_=wk_r[:, :, 0:P])
        for cc in range(5, 8):
            nc.gpsimd.dma_start(out=xt_sb[:, cc, 0:NI], in_=xt_r[:, cc, 0:NI])
        for cc in range(3):
            nc.gpsimd.dma_start(out=yt_sb[:, cc, 0:NI], in_=yt_r[:, cc, 0:NI])
        for cc in range(5, 8):
            nc.gpsimd.dma_start(out=xt_sb[:, cc, NI:L], in_=xt_r[:, cc, NI:L])
        for cc in range(3):
            nc.gpsimd.dma_start(out=yt_sb[:, cc, NI:L], in_=yt_r[:, cc, NI:L])
        for cc in range(3, 6):
            nc.gpsimd.dma_start(out=wv_sb[:, cc], in_=wv_r[:, cc])
        nc.gpsimd.dma_start(out=wk_sb[:, :, P:DL], in_=wk_r[:, :, P:DL])
        # scalar
        for cc in range(3, 8):
            nc.scalar.dma_start(out=yt_sb[:, cc, 0:NI], in_=yt_r[:, cc, 0:NI])
        for cc in range(3, 8):
            nc.scalar.dma_start(out=yt_sb[:, cc, NI:L], in_=yt_r[:, cc, NI:L])
        for cc in range(6, 8):
            nc.scalar.dma_start(out=wv_sb[:, cc], in_=wv_r[:, cc])
        for dt in range(4):
            nc.scalar.dma_start(out=wo_sb[:, dt], in_=wo_r[:, dt])

        # ones blocks of Vaug only (even h: cols DH:P, odd h: cols 0:DH)
        va_r = va_sb.rearrange("p it (h s) -> p it h s", s=P)
        nc.vector.memset(va_r[:, :, 0::2, DH:P], 1.0)
        nc.gpsimd.memset(va_r[:, :, 1::2, 0:DH], 1.0)
        # f32 identity for the tail partition-move matmuls
        id_sb = persist.tile([P, P], F32, tag="ident")
        make_identity(nc, id_sb)

        # ---- HAM warm-up: dummy matmuls while input DMA streams in ----
        ps_dummy = ps_half.tile([P, NI], F32, tag="sth")
        for _ in range(N_DUMMY):
            nc.tensor.matmul(
                ps_dummy[:], dummy_sb[:, 0:P], dummy_sb[:], start=True, stop=True
            )

        scale = DH**-0.5

        # ---- chain emitters (each a short burst of independent PE work) ----

        def proj_chain(w_sb, t_sb, rhs_sb, dt, ic):
            ps = ps_acc.tile([P, NI], F32, tag="acc")
            for cc in range(CC):
                nc.tensor.matmul(
                    ps[:],
                    w_sb[:, cc, dt * P : (dt + 1) * P],
                    rhs_sb[:, cc, ic * NI : (ic + 1) * NI],
                    start=(cc == 0),
                    stop=(cc == CC - 1),
                )
            nc.vector.tensor_copy(t_sb[:, dt, ic * NI : (ic + 1) * NI], ps[:])

        def v_chain(jt):
            ps = ps_acc.tile([P, NI], F32, tag="acc")
            for cc in range(CC):
                nc.tensor.matmul(
                    ps[:],
                    yt_sb[:, cc, jt * P : (jt + 1) * P],
                    wv_sb[:, cc, :],
                    start=(cc == 0),
                    stop=(cc == CC - 1),
                )
            va_blk = va_sb[:, jt].rearrange("p (h s) -> p h s", s=P)
            ps_blk = ps.rearrange("p (h s) -> p h s", s=DH)
            nc.vector.tensor_copy(va_blk[:, 0::2, 0:DH], ps_blk[:, 0::2, :])
            nc.vector.tensor_copy(va_blk[:, 1::2, DH:P], ps_blk[:, 1::2, :])

        # Deferred finishers: the normalize needs a partition move between
        # two DVE ops; emitting the post-move ops immediately would stall
        # the in-order DVE stream for the move latency. Each ctx chain
        # queues its post-move ops and the next fill slot flushes them.
        deferred = []

        def flush_deferred():
            while deferred:
                deferred.pop(0)()

        def ctx_chain(h, ptile, ic, fast_norm=False):
            dt, r0 = divmod(h * DH, P)
            ct = ps_acc.tile([P, NI], F32, tag="acc")
            for jt in range(IT):
                nc.tensor.matmul(
                    ct[:],
                    va_sb[:, jt, h * P : (h + 1) * P],
                    ptile[:, jt, ic * NI : (ic + 1) * NI],
                    start=(jt == 0),
                    stop=(jt == IT - 1),
                )
            # custom DVE ops (reciprocal) only work at base partition 0, and
            # ctx/rowsum land on complementary partition halves. Mid-kernel
            # the move goes through a SBUF->SBUF DMA whose round-trip hides
            # under later fill slots; at the tail (fast_norm) an identity
            # matmul does the partition move with ~2us less latency at the
            # cost of a PE slot and a PSUM bank.
            rc = small.tile([P, NI], F32, tag="rc")
            if r0 == 0:
                rs = small.tile([P, NI], F32, tag="rs")
                nc.vector.tensor_copy(rs[DH:P, :], ct[DH:P, :])
                if fast_norm:
                    pr = ps_acc.tile([P, NI], F32, tag="acc")
                    nc.tensor.matmul(
                        pr[0:DH, :], id_sb[DH:P, DH:P], rs[DH:P, :],
                        start=True, stop=True,
                    )

                    def fin():
                        nc.vector.reciprocal_approx_fast(rc[0:DH, :], pr[0:DH, :])
                        nc.vector.tensor_mul(
                            cx_sb[0:DH, dt, ic * NI : (ic + 1) * NI],
                            ct[0:DH, :],
                            rc[0:DH, :],
                        )
                else:
                    nc.gpsimd.dma_start(out=rs[0:DH, :], in_=rs[DH:P, :])

                    def fin():
                        nc.vector.reciprocal_approx_fast(rc[0:DH, :], rs[0:DH, :])
                        nc.vector.tensor_mul(
                            cx_sb[0:DH, dt, ic * NI : (ic + 1) * NI],
                            ct[0:DH, :],
                            rc[0:DH, :],
                        )
            else:
                nc.vector.reciprocal_approx_fast(rc[0:DH, :], ct[0:DH, :])
                if fast_norm:
                    pr = ps_acc.tile([P, NI], F32, tag="acc")
                    nc.tensor.matmul(
                        pr[DH:P, :], id_sb[0:DH, 0:DH], rc[0:DH, :],
                        start=True, stop=True,
                    )
                    # DVE can read only one PSUM operand per instruction
                    rc2 = small.tile([P, NI], F32, tag="rc2")
                    nc.vector.tensor_copy(rc2[DH:P, :], pr[DH:P, :])

                    def fin():
                        nc.vector.tensor_mul(
                            cx_sb[DH:P, dt, ic * NI : (ic + 1) * NI],
                            ct[DH:P, :],
                            rc2[DH:P, :],
                        )
                else:
                    nc.gpsimd.dma_start(out=rc[DH:P, :], in_=rc[0:DH, :])

                    def fin():
                        nc.vector.tensor_mul(
                            cx_sb[DH:P, dt, ic * NI : (ic + 1) * NI],
                            ct[DH:P, :],
                            rc[DH:P, :],
                        )

            deferred.append(fin)

        def po_chain(it, oc, dts, out_ap, o_st=None, copy_eng="vector"):
            # out-projection partial over the given d-tiles; if o_st is
            # given, drain into the caller's staging tile (no DMA here)
            po = ps_acc.tile([P, NI], F32, tag="acc")
            for k, dt in enumerate(dts):
                nc.tensor.matmul(
                    po[:],
                    cx_sb[:, dt, it * P : (it + 1) * P],
                    wo_sb[:, dt, oc * NI : (oc + 1) * NI],
                    start=(k == 0),
                    stop=(k == len(dts) - 1),
                )
            if o_st is None:
                o_st_l = small.tile([P, NI], BF16, tag="ost")
                nc.vector.tensor_copy(o_st_l[:], po[:])
                out_r = out_ap.rearrange("(it p) o -> it p o", p=P)
                nc.sync.dma_start(
                    out=out_r[it, :, oc * NI : (oc + 1) * NI], in_=o_st_l[:]
                )
            else:
                dst = o_st[:, oc * NI : (oc + 1) * NI]
                if copy_eng == "scalar":
                    nc.scalar.copy(dst, po[:])
                else:
                    nc.vector.tensor_copy(dst, po[:])

        def po_pair(it, dts, out_ap, copy_eng, dma_eng):
            # both oc halves drained into one staging tile -> single 256KB
            # DMA; drains on the given engine (scalar engine is idle at the
            # tail once the exp stream is done)
            o_st = small.tile([P, 2 * NI], BF16, tag="ostw")
            for oc in range(NIC):
                po_chain(it, oc, dts, out_ap, o_st=o_st, copy_eng=copy_eng)
            out_r = out_ap.rearrange("(it p) o -> it p o", p=P)
            dma_eng(out=out_r[it], in_=o_st[:])

        # ---- ST + exp for a head pair in four (ic, jt-range) quarters ----

        def st_pair(hp, fills):
            # Each (ic, jt) step: one MM per head into its own 1-bank PSUM
            # tile; the heads run concurrently in different PE row groups.
            dt = hp
            ptiles = []
            for h_off in range(2):
                pt_tile = pt_pool.tile([P, IT, L], BF16, tag="pt")
                ptiles.append(pt_tile)
            fills = list(fills)
            step = 0
            for ic, j0 in ((0, 0), (1, 0), (0, 4), (1, 4)):
                for jt in range(j0, j0 + 4):
                    halves = [
                        ps_half.tile([P, NI], F32, tag="sth", name=f"sth{h_off}")
                        for h_off in range(2)
                    ]
                    for h_off in range(2):
                        r0 = DH * h_off
                        nc.tensor.matmul(
                            halves[h_off][:],
                            kt_sb[r0 : r0 + DH, dt, jt * P : (jt + 1) * P],
                            qt_sb[r0 : r0 + DH, dt, ic * NI : (ic + 1) * NI],
                            start=True,
                            stop=True,
                        )
                    for h_off in range(2):
                        nc.scalar.activation(
                            ptiles[h_off][:, jt, ic * NI : (ic + 1) * NI],
                            halves[h_off][:],
                            mybir.ActivationFunctionType.Exp,
                            scale=scale,
                        )
                    if step < len(fills) and fills[step]:
                        pending = list(deferred)
                        deferred.clear()
                        for f in fills[step]:
                            f()
                        for f in pending:
                            f()
                    step += 1
            return ptiles

        # ---- schedule ----
        mk = lambda f, *a: (lambda: f(*a))
        E = None

        # only the ic0 halves of the dt0 projections gate the first score
        # quarter; the ic1 halves are early fills inside pair 0
        proj_chain(wq_sb, qt_sb, xt_sb, 0, 0)
        proj_chain(wk_sb, kt_sb, yt_sb, 0, 0)

        # pair 0 quarters: (ic0,jt0-3) needs only the prefix; QT0-ic1 before
        # step 4 (= quarter B); KT0-ic1 before step 8 (ST jt4 of quarter C).
        pt0 = st_pair(
            0,
            [
                E, E, E, [mk(proj_chain, wq_sb, qt_sb, xt_sb, 0, 1)],
                E, E, E, [mk(proj_chain, wk_sb, kt_sb, yt_sb, 0, 1)],
                E, [mk(v_chain, 0)], E, [mk(v_chain, 1)],
                E, [mk(v_chain, 2)], E, [mk(v_chain, 3)],
            ],
        )
        # QT1/KT1 cover pair-0's exp drain
        proj_chain(wq_sb, qt_sb, xt_sb, 1, 0)
        proj_chain(wq_sb, qt_sb, xt_sb, 1, 1)
        proj_chain(wk_sb, kt_sb, yt_sb, 1, 0)

        # pair 1: remaining V chains first (ctx h0 reads all of va), KT1-ic1
        # before step 8, then ctx of heads 0/1
        pt1 = st_pair(
            1,
            [
                [mk(v_chain, 4)], E, [mk(v_chain, 5)], E,
                [mk(v_chain, 6)], [mk(v_chain, 7)],
                [mk(proj_chain, wk_sb, kt_sb, yt_sb, 1, 1)], E,
                [mk(ctx_chain, 0, pt0[0], 0)], E,
                [mk(ctx_chain, 0, pt0[0], 1)], E,
                [mk(ctx_chain, 1, pt0[1], 0)], E,
                [mk(ctx_chain, 1, pt0[1], 1)], E,
            ],
        )
        proj_chain(wq_sb, qt_sb, xt_sb, 2, 0)
        proj_chain(wq_sb, qt_sb, xt_sb, 2, 1)
        proj_chain(wk_sb, kt_sb, yt_sb, 2, 0)

        # pair 2: KT2-ic1 before step 8, ctx of heads 2/3, QT3/KT3
        pt2 = st_pair(
            2,
            [
                [mk(ctx_chain, 2, pt1[0], 0)], E,
                [mk(ctx_chain, 2, pt1[0], 1)], E,
                [mk(proj_chain, wk_sb, kt_sb, yt_sb, 2, 1)], E,
                [mk(ctx_chain, 3, pt1[1], 0)], E,
                [mk(ctx_chain, 3, pt1[1], 1)], E,
                [mk(proj_chain, wq_sb, qt_sb, xt_sb, 3, 0)], E,
                [mk(proj_chain, wq_sb, qt_sb, xt_sb, 3, 1)], E,
                [mk(proj_chain, wk_sb, kt_sb, yt_sb, 3, 0)], E,
            ],
        )
        proj_chain(wk_sb, kt_sb, yt_sb, 3, 1)

        # pair 3: ctx of heads 4/5, then out-proj partial A over dt 0..2
        # (heads 0..5). poA[12:16] stays after the st loop to cover the
        # final exp drain that the tail ctx chains must wait for.
        poA = [
            mk(po_chain, it, oc, (0, 1, 2), out_a)
            for it in range(IT)
            for oc in range(NIC)
        ]
        pt3 = st_pair(
            3,
            [
                [mk(ctx_chain, 4, pt2[0], 0)], E,
                [mk(ctx_chain, 4, pt2[0], 1)], E,
                [mk(ctx_chain, 5, pt2[1], 0)], E,
                [mk(ctx_chain, 5, pt2[1], 1)], E,
                poA[0:2], E, poA[2:4], poA[4:6],
                poA[6:8], poA[8:10], poA[10:12], E,
            ],
        )
        for f in poA[12:16]:
            f()
        flush_deferred()

        # tail: ctx chains 60/70/71 first (their normalize moves hide under
        # later work), out blocks it0..3 as soon as fin60/fin70 flush, then
        # ctx61 and the rest. Drains alternate scalar/vector; output DMAs
        # split across two queues.
        def otail(it):
            po_pair(
                it, (3,), out_c,
                "scalar" if it % 2 == 0 else "vector",
                nc.sync.dma_start if it % 2 == 0 else nc.gpsimd.dma_start,
            )

        ctx_chain(6, pt3[0], 0)          # queues fin60
        pending = list(deferred)
        deferred.clear()
        ctx_chain(7, pt3[1], 0)          # queues fin70
        for f in pending:                # fin60
            f()
        pending = list(deferred)
        deferred.clear()
        ctx_chain(7, pt3[1], 1, fast_norm=True)   # queues fin71
        for f in pending:                # fin70
            f()
        otail(0)                         # needs fin60/fin70 only
        otail(1)
        pending = list(deferred)
        deferred.clear()
        ctx_chain(6, pt3[0], 1, fast_norm=True)   # queues fin61
        for f in pending:                # fin71
            f()
        otail(2)
        otail(3)
        flush_deferred()                 # fin61
        for it in range(IT // 2, IT):
            otail(it)


_NC_CACHE = None


def _get_nc():
    global _NC_CACHE
    if _NC_CACHE is None:
        _NC_CACHE = _build_kernel()
    return _NC_CACHE


def kernel(x, y, Wq, Wk, Wv, Wo, _trace=False):
    bf = ml_dtypes.bfloat16
    x = np.asarray(x, np.float32)
    y = np.asarray(y, np.float32)
    xtb = [np.ascontiguousarray(np.asarray(x[b]).T).astype(bf) for b in range(B)]
    ytb = [np.ascontiguousarray(np.asarray(y[b]).T).astype(bf) for b in range(B)]
    wqs = [np.ascontiguousarray(np.asarray(Wq)[:, t * DL : (t + 1) * DL]).astype(bf) for t in range(TP)]
    wks = [np.ascontiguousarray(np.asarray(Wk)[:, t * DL : (t + 1) * DL]).astype(bf) for t in range(TP)]
    wvs = [np.ascontiguousarray(np.asarray(Wv)[:, t * DL : (t + 1) * DL]).astype(bf) for t in range(TP)]
    wos = [np.ascontiguousarray(np.asarray(Wo)[t * DL : (t + 1) * DL, :]).astype(bf) for t in range(TP)]

    in_maps = []
    for b in range(B):
        for t in range(TP):
            in_maps.append(
                {
                    "xt": xtb[b],
                    "yt": ytb[b],
                    "wq": wqs[t],
                    "wk": wks[t],
                    "wv": wvs[t],
                    "wo": wos[t],
                }
            )

    nc = _get_nc()
    res = run_bass_kernel_spmd(
        nc, in_maps, core_ids=list(range(N_CORES)), trace=_trace
    )
    out = np.empty((B, L, U), np.float32)
    for b in range(B):
        out[b] = (
            np.asarray(res.results[2 * b]["out_a"], np.float32)
            + np.asarray(res.results[2 * b]["out_c"], np.float32)
            + np.asarray(res.results[2 * b + 1]["out_a"], np.float32)
            + np.asarray(res.results[2 * b + 1]["out_c"], np.float32)
        )
    if _trace:
        return out, res
    return out


# revision 28
# speedup vs baseline: 1.0349x; 1.0349x over previous
"""Multi-head attention (B=4, L=1024, D=1024, H=16, DH=64) on 8 TRN2 NeuronCores.

Sharding: data-parallel over batch (4) x tensor-parallel over heads (2).
Core c = 2*b + t computes, for batch b, heads [t*8, (t+1)*8):
    QT = Wq_t^T X^T, KT = Wk_t^T X^T, V = Y Wv_t        (all bf16 matmuls)
    per head: S^T = K_h Q_h^T; P^T = exp(S^T/8);
              [ctx^T; rowsum] = Vaug_h^T P^T;  ctxn = ctx / rowsum
    O_partial = ctxn^T Wo_t                              (f32, two dt-halves)
Host pre-transposes X/Y, casts to bf16, and sums the four f32 partials
(2 tensor-parallel cores x 2 dt-halves) per batch.

Schedule notes (engines execute their streams in order, so emission order is
the software pipeline):
  - Input DMAs are spread across 4 engine queues (sync/vector/gpsimd/scalar)
    with the critical prefix (wq/wk dt0 columns, xt, yt ic0-half) issued
    first on each queue; issuing everything from one queue serializes at
    ~0.6us per descriptor-gen instruction.
  - ~20 dummy matmuls run during the unavoidable input-DMA wait so the PE
    HAM clock-gate warms up (K=8/8) before real matmuls start; otherwise the
    first ~20us of matmuls run at 1.2GHz instead of 2.4GHz.
  - Score matmuls for the head pair are emitted interleaved (h0,h1,h0,h1);
    the two heads' K blocks sit at partitions 0:64 / 64:128 so the MMs land
    in different PE row groups and run concurrently (~2x on the ST phase).
  - Every ST (scores) step, whose exp drain on the scalar engine is slower
    than the matmuls, is followed by an independent fill chain (V projection,
    next d-tile QT/KT, an earlier head's ctx, or an out-projection partial)
    so the tensor engine never waits for the scalar engine.
  - out_a (Wo over dt 0..2) drains during the last score phase; out_c (dt3)
    uses 2-bank-wide PSUM chains at the tail with drains alternating between
    the scalar and vector engines and output DMAs split over two queues.
"""

import numpy as np
import ml_dtypes

import concourse.tile as tile
import concourse.mybir as mybir
from concourse import bacc
from concourse.bass_utils import run_bass_kernel_spmd

B, L, D, U, H = 4, 1024, 1024, 1024, 16
DH = U // H          # 64 head dim
TP = 2               # tensor-parallel ways (heads)
DL = U // TP         # 512 local units
HL = H // TP         # 8 local heads
P = 128              # partitions
NI = 512             # matmul free-dim chunk (one PSUM bank of f32)
CC = D // P          # 8 contraction chunks for projections
DT = DL // P         # 4 local d-tiles
IT = L // P          # 8 i/j tiles
NIC = L // NI        # 2 free chunks of 512
N_CORES = 8
N_DUMMY = 35         # HAM warm-up matmuls during the input-DMA wait

BF16 = mybir.dt.bfloat16
F32 = mybir.dt.float32
F8 = mybir.dt.float8e4


def _build_kernel():
    nc = bacc.Bacc(
        "TRN2", target_bir_lowering=False, debug=False, num_devices=N_CORES
    )
    xt = nc.dram_tensor("xt", [D, L], BF16, kind="ExternalInput").ap()
    yt = nc.dram_tensor("yt", [D, L], BF16, kind="ExternalInput").ap()
    wq = nc.dram_tensor("wq", [D, DL], BF16, kind="ExternalInput").ap()
    wk = nc.dram_tensor("wk", [D, DL], BF16, kind="ExternalInput").ap()
    wv = nc.dram_tensor("wv", [D, DL], BF16, kind="ExternalInput").ap()
    wo = nc.dram_tensor("wo", [DL, U], BF16, kind="ExternalInput").ap()
    out_a = nc.dram_tensor("out_a", [L, U], BF16, kind="ExternalOutput").ap()
    out_c = nc.dram_tensor("out_c", [L, U], BF16, kind="ExternalOutput").ap()

    with tile.TileContext(nc) as tc:
        _mha_body(tc, out_a, out_c, xt, yt, wq, wk, wv, wo)

    nc.compile()
    return nc


def _mha_body(tc, out_a, out_c, xt, yt, wq, wk, wv, wo):
    nc = tc.nc
    from contextlib import ExitStack

    with ExitStack() as ctx:
        persist = ctx.enter_context(tc.tile_pool(name="persist", bufs=1))
        pt_pool = ctx.enter_context(tc.tile_pool(name="pt", bufs=4))
        # ST tiles: [P, 1024] f32 = 2 banks each
        ps_wide = ctx.enter_context(tc.tile_pool(name="ps_wide", bufs=2, space="PSUM"))
        # single-bank accumulators (projections, V, ctx, out-proj)
        ps_acc = ctx.enter_context(tc.tile_pool(name="ps_acc", bufs=4, space="PSUM"))
        small = ctx.enter_context(tc.tile_pool(name="small", bufs=4))

        # persistent SBUF tensors (activations/QKV weights in fp8: the
        # projections run DoubleRow with 256-deep contraction per step)
        xt_sb = persist.tile([P, CC, L], BF16, tag="xt")
        yt_sb = persist.tile([P, CC, L], BF16, tag="yt")
        wq_sb = persist.tile([P, CC, DL], BF16, tag="wq")
        wk_sb = persist.tile([P, CC, DL], BF16, tag="wk")
        wv_sb = persist.tile([P, CC, DL], BF16, tag="wv")
        wo_sb = persist.tile([P, DT, U], BF16, tag="wo")
        qt_sb = persist.tile([P, DT, L], BF16, tag="qt")
        kt_sb = persist.tile([P, DT, L], BF16, tag="kt")
        # Vaug: per j-chunk, per head a 128-col block; even h: [V_h | ones],
        # odd h: [ones | V_h] (ctx^T lands on the head's own cx partitions)
        va_sb = persist.tile([P, IT, HL * P], BF16, tag="va")
        cx_sb = persist.tile([P, DT, L], BF16, tag="cx")
        dummy_sb = persist.tile([P, NI], BF16, tag="dummy")

        wq_r = wq.rearrange("(cc p) d -> p cc d", p=P)
        wk_r = wk.rearrange("(cc p) d -> p cc d", p=P)
        wv_r = wv.rearrange("(cc p) d -> p cc d", p=P)
        xt_r = xt.rearrange("(cc p) i -> p cc i", p=P)
        yt_r = yt.rearrange("(cc p) i -> p cc i", p=P)
        wo_r = wo.rearrange("(dt p) o -> p dt o", p=P)

        # ---- input DMAs: 3 hw queues (SP/gpsimd/Activation). Class-1
        # (wq/wk dt0 cols, xt, yt ic0 — gates the first score phase) is
        # split evenly across the queue heads; class-2 strictly behind.
        nc.vector.memset(dummy_sb[:], 1.0)
        # class1 gates the first score phase; class2 (yt-ic1, wv) gates the
        # pair-0 fill chains; class3 (wq/wk rest) gates pair-1; wo is last.
        # sync: wq-dt0, xt 0..2, yt-ic0 0..1 | yt-ic1 0..3, wv 0..1 | ...
        nc.sync.dma_start(out=wq_sb[:, :, 0:P], in_=wq_r[:, :, 0:P])
        for cc in range(3):
            nc.sync.dma_start(out=xt_sb[:, cc], in_=xt_r[:, cc])
        for cc in range(2):
            nc.sync.dma_start(out=yt_sb[:, cc, 0:NI], in_=yt_r[:, cc, 0:NI])
        for cc in range(4):
            nc.sync.dma_start(out=yt_sb[:, cc, NI:L], in_=yt_r[:, cc, NI:L])
        for cc in range(2):
            nc.sync.dma_start(out=wv_sb[:, cc], in_=wv_r[:, cc])
        nc.sync.dma_start(out=wq_sb[:, :, P:DL], in_=wq_r[:, :, P:DL])
        for dt in range(2):
            nc.sync.dma_start(out=wo_sb[:, dt], in_=wo_r[:, dt])
        # gpsimd: wk-dt0, xt 3..5, yt-ic0 2..3 | yt-ic1 4..7, wv 2..3 | ...
        nc.gpsimd.dma_start(out=wk_sb[:, :, 0:P], in_=wk_r[:, :, 0:P])
        for cc in range(3, 6):
            nc.gpsimd.dma_start(out=xt_sb[:, cc], in_=xt_r[:, cc])
        for cc in range(2, 4):
            nc.gpsimd.dma_start(out=yt_sb[:, cc, 0:NI], in_=yt_r[:, cc, 0:NI])
        for cc in range(4, 8):
            nc.gpsimd.dma_start(out=yt_sb[:, cc, NI:L], in_=yt_r[:, cc, NI:L])
        for cc in range(2, 4):
            nc.gpsimd.dma_start(out=wv_sb[:, cc], in_=wv_r[:, cc])
        nc.gpsimd.dma_start(out=wk_sb[:, :, P:DL], in_=wk_r[:, :, P:DL])
        for dt in range(2, 4):
            nc.gpsimd.dma_start(out=wo_sb[:, dt], in_=wo_r[:, dt])
        # scalar: xt 6..7, yt-ic0 4..7 | wv 4..7
        for cc in range(6, 8):
            nc.scalar.dma_start(out=xt_sb[:, cc], in_=xt_r[:, cc])
        for cc in range(4, 8):
            nc.scalar.dma_start(out=yt_sb[:, cc, 0:NI], in_=yt_r[:, cc, 0:NI])
        for cc in range(4, 8):
            nc.scalar.dma_start(out=wv_sb[:, cc], in_=wv_r[:, cc])

        # ones blocks of Vaug only (even h: cols DH:P, odd h: cols 0:DH)
        va_r = va_sb.rearrange("p it (h s) -> p it h s", s=P)
        nc.vector.memset(va_r[:, :, 0::2, DH:P], 1.0)
        nc.gpsimd.memset(va_r[:, :, 1::2, 0:DH], 1.0)
        # f32 identity for the tail partition-move matmuls
        from concourse.masks import make_identity

        id_sb = persist.tile([P, P], F32, tag="ident")
        make_identity(nc, id_sb)

        # ---- HAM warm-up: dummy matmuls while input DMA streams in ----
        ps_dummy = ps_wide.tile([P, 2 * NI], F32, tag="wide")
        for _ in range(N_DUMMY):
            nc.tensor.matmul(
                ps_dummy[:, 0:NI], dummy_sb[:, 0:P], dummy_sb[:], start=True, stop=True
            )

        scale = DH**-0.5

        # ---- chain emitters (each a short burst of independent PE work) ----

        DR = mybir.MatmulPerfMode.DoubleRow

        def proj_chain(w_sb, t_sb, rhs_sb, dt, ic):
            ps = ps_acc.tile([P, NI], F32, tag="acc")
            for cc in range(CC):
                nc.tensor.matmul(
                    ps[:],
                    w_sb[:, cc, dt * P : (dt + 1) * P],
                    rhs_sb[:, cc, ic * NI : (ic + 1) * NI],
                    start=(cc == 0),
                    stop=(cc == CC - 1),
                )
            nc.vector.tensor_copy(t_sb[:, dt, ic * NI : (ic + 1) * NI], ps[:])

        def proj_chain2(w_sb, t_sb, rhs_sb, dt):
            # both ic halves in one emission: consecutive matmuls share the
            # stationary operand (bass dedupes the LDWEIGHTS) and there's a
            # single chain-boundary weight-load exposure instead of two
            ps0 = ps_acc.tile([P, NI], F32, tag="acc")
            ps1 = ps_acc.tile([P, NI], F32, tag="acc")
            for cc in range(CC):
                for ic, ps in ((0, ps0), (1, ps1)):
                    nc.tensor.matmul(
                        ps[:],
                        w_sb[:, cc, dt * P : (dt + 1) * P],
                        rhs_sb[:, cc, ic * NI : (ic + 1) * NI],
                        start=(cc == 0),
                        stop=(cc == CC - 1),
                    )
            nc.vector.tensor_copy(t_sb[:, dt, 0:NI], ps0[:])
            nc.vector.tensor_copy(t_sb[:, dt, NI:L], ps1[:])

        def v_chain(jt):
            ps = ps_acc.tile([P, NI], F32, tag="acc")
            for cc in range(CC):
                nc.tensor.matmul(
                    ps[:],
                    yt_sb[:, cc, jt * P : (jt + 1) * P],
                    wv_sb[:, cc, :],
                    start=(cc == 0),
                    stop=(cc == CC - 1),
                )
            va_blk = va_sb[:, jt].rearrange("p (h s) -> p h s", s=P)
            ps_blk = ps.rearrange("p (h s) -> p h s", s=DH)
            nc.vector.tensor_copy(va_blk[:, 0::2, 0:DH], ps_blk[:, 0::2, :])
            nc.vector.tensor_copy(va_blk[:, 1::2, DH:P], ps_blk[:, 1::2, :])

        # Deferred finishers: the normalize needs a SBUF->SBUF DMA between
        # two DVE ops; emitting the post-DMA ops immediately would stall the
        # in-order DVE stream (and the PSUM-releasing copies queued behind
        # it) for the DMA round-trip. Instead each ctx chain queues its
        # post-DMA ops and the next fill slot flushes them.
        deferred = []

        def flush_deferred():
            while deferred:
                deferred.pop(0)()

        def ctx_chain(h, ptile, ic, fast_norm=False):
            dt, r0 = divmod(h * DH, P)
            ct = ps_acc.tile([P, NI], F32, tag="acc")
            for jt in range(IT):
                nc.tensor.matmul(
                    ct[:],
                    va_sb[:, jt, h * P : (h + 1) * P],
                    ptile[:, jt, ic * NI : (ic + 1) * NI],
                    start=(jt == 0),
                    stop=(jt == IT - 1),
                )
            # custom DVE ops (reciprocal) only work at base partition 0, and
            # ctx/rowsum land on complementary partition halves. Mid-kernel
            # the move goes through a SBUF->SBUF DMA whose round-trip hides
            # under later fill slots; at the tail (fast_norm) a tiny
            # identity matmul does the partition move with ~1us less
            # latency at the cost of a PE slot and a PSUM bank.
            rc = small.tile([P, NI], F32, tag="rc")
            if r0 == 0:
                rs = small.tile([P, NI], F32, tag="rs")
                nc.vector.tensor_copy(rs[DH:P, :], ct[DH:P, :])
                if fast_norm:
                    pr = ps_acc.tile([P, NI], F32, tag="acc")
                    nc.tensor.matmul(
                        pr[0:DH, :], id_sb[DH:P, DH:P], rs[DH:P, :],
                        start=True, stop=True,
                    )

                    def fin():
                        nc.vector.reciprocal_approx_fast(rc[0:DH, :], pr[0:DH, :])
                        nc.vector.tensor_mul(
                            cx_sb[0:DH, dt, ic * NI : (ic + 1) * NI],
                            ct[0:DH, :],
                            rc[0:DH, :],
                        )
                else:
                    nc.gpsimd.dma_start(out=rs[0:DH, :], in_=rs[DH:P, :])

                    def fin():
                        nc.vector.reciprocal_approx_fast(rc[0:DH, :], rs[0:DH, :])
                        nc.vector.tensor_mul(
                            cx_sb[0:DH, dt, ic * NI : (ic + 1) * NI],
                            ct[0:DH, :],
                            rc[0:DH, :],
                        )
            else:
                nc.vector.reciprocal_approx_fast(rc[0:DH, :], ct[0:DH, :])
                if fast_norm:
                    pr = ps_acc.tile([P, NI], F32, tag="acc")
                    nc.tensor.matmul(
                        pr[DH:P, :], id_sb[0:DH, 0:DH], rc[0:DH, :],
                        start=True, stop=True,
                    )
                    # DVE can read only one PSUM operand per instruction
                    rc2 = small.tile([P, NI], F32, tag="rc2")
                    nc.vector.tensor_copy(rc2[DH:P, :], pr[DH:P, :])

                    def fin():
                        nc.vector.tensor_mul(
                            cx_sb[DH:P, dt, ic * NI : (ic + 1) * NI],
                            ct[DH:P, :],
                            rc2[DH:P, :],
                        )
                else:
                    nc.gpsimd.dma_start(out=rc[DH:P, :], in_=rc[0:DH, :])

                    def fin():
                        nc.vector.tensor_mul(
                            cx_sb[DH:P, dt, ic * NI : (ic + 1) * NI],
                            ct[DH:P, :],
                            rc[DH:P, :],
                        )

            deferred.append(fin)

        def po_chain(it, oc, dts, out_ap):
            # out-projection partial over the given d-tiles
            po = ps_acc.tile([P, NI], F32, tag="acc")
            for k, dt in enumerate(dts):
                nc.tensor.matmul(
                    po[:],
                    cx_sb[:, dt, it * P : (it + 1) * P],
                    wo_sb[:, dt, oc * NI : (oc + 1) * NI],
                    start=(k == 0),
                    stop=(k == len(dts) - 1),
                )
            o_st = small.tile([P, NI], BF16, tag="ost")
            nc.vector.tensor_copy(o_st[:], po[:])
            out_r = out_ap.rearrange("(it p) o -> it p o", p=P)
            nc.sync.dma_start(
                out=out_r[it, :, oc * NI : (oc + 1) * NI], in_=o_st[:]
            )

        def po_wide_chain(it, dts, out_ap, copy_eng, dma_eng):
            # both oc halves in one 2-bank psum tile (the ST pool slots are
            # idle once the score phases are done); single FD=1024 bf16
            # drain + one 256KB DMA
            po = ps_wide.tile([P, 2 * NI], F32, tag="wide")
            for oc in range(NIC):
                for k, dt in enumerate(dts):
                    nc.tensor.matmul(
                        po[:, oc * NI : (oc + 1) * NI],
                        cx_sb[:, dt, it * P : (it + 1) * P],
                        wo_sb[:, dt, oc * NI : (oc + 1) * NI],
                        start=(k == 0),
                        stop=(k == len(dts) - 1),
                    )
            o_st = small.tile([P, 2 * NI], BF16, tag="ostw")
            if copy_eng == "scalar":
                nc.scalar.copy(o_st[:], po[:])
            else:
                nc.vector.tensor_copy(o_st[:], po[:])
            out_r = out_ap.rearrange("(it p) o -> it p o", p=P)
            dma_eng(out=out_r[it], in_=o_st[:])

        # ---- ST + exp for a head pair, fill chains between steps ----
        # The two heads' score MMs are interleaved so they run in different
        # PE row groups (partitions 0:64 vs 64:128) concurrently.

        def st_pair(hp, fills):
            dt = hp
            ptiles = []
            for h_off in range(2):
                pt_tile = pt_pool.tile([P, IT, L], BF16, tag="pt")
                ptiles.append(pt_tile)
            fills = list(fills)
            for jt in range(IT):
                sts = [
                    ps_wide.tile([P, 2 * NI], F32, tag="wide", name=f"st{h_off}")
                    for h_off in range(2)
                ]
                # h-outer/ic-inner: bass dedupes the LDWEIGHTS across the
                # two ic matmuls of a head, and h1's weight load pulls
                # ahead into its own row group during h0's matmuls, so
                # h1-ic0 runs concurrently with h0-ic1 (3 slots per step).
                # Alternating h0/h1 would force a weight reload per MM
                # that serializes against the in-flight MM's row group.
                for h_off in range(2):
                    r0 = DH * h_off
                    for ic in range(NIC):
                        nc.tensor.matmul(
                            sts[h_off][:, ic * NI : (ic + 1) * NI],
                            kt_sb[r0 : r0 + DH, dt, jt * P : (jt + 1) * P],
                            qt_sb[r0 : r0 + DH, dt, ic * NI : (ic + 1) * NI],
                            start=True,
                            stop=True,
                        )
                for h_off in range(2):
                    nc.scalar.activation(
                        ptiles[h_off][:, jt, :],
                        sts[h_off][:],
                        mybir.ActivationFunctionType.Exp,
                        scale=scale,
                    )
                if jt < len(fills):
                    pending = list(deferred)
                    deferred.clear()
                    for f in fills[jt]:
                        f()
                    for f in pending:
                        f()
            return ptiles

        # ---- schedule ----
        mk = lambda f, *a: (lambda: f(*a))

        # dt0 projections gate the first score phase; only these run before
        # pair 0 (everything else becomes fill work). K's ic1 half stays a
        # fill: it needs yt-ic1 (class-2 DMA) and is only read by ST jt4.
        proj_chain2(wq_sb, qt_sb, xt_sb, 0)
        proj_chain(wk_sb, kt_sb, yt_sb, 0, 0)

        # pair 0: fill with K-dt0-ic1, V chains and QT1/KT1
        pt0 = st_pair(
            0,
            [
                [mk(proj_chain, wk_sb, kt_sb, yt_sb, 0, 1)],
                [mk(v_chain, 0)],
                [mk(v_chain, 1)],
                [mk(v_chain, 2)],
                [mk(v_chain, 3)],
                [mk(proj_chain, wq_sb, qt_sb, xt_sb, 1, 0)],
                [mk(proj_chain, wq_sb, qt_sb, xt_sb, 1, 1)],
                [mk(proj_chain, wk_sb, kt_sb, yt_sb, 1, 0)],
            ],
        )
        # KT1-ic1 covers pair-0's exp drain (needed by pair-1 ST jt4)
        proj_chain(wk_sb, kt_sb, yt_sb, 1, 1)

        # pair 1: remaining V chains first (ctx h0 reads all of va), then
        # ctx of heads 0/1 and QT2
        pt1 = st_pair(
            1,
            [
                [mk(v_chain, 4)],
                [mk(v_chain, 5)],
                [mk(v_chain, 6)],
                [mk(v_chain, 7)],
                [mk(ctx_chain, 0, pt0[0], 0)],
                [mk(ctx_chain, 0, pt0[0], 1)],
                [mk(ctx_chain, 1, pt0[1], 0)],
                [mk(ctx_chain, 1, pt0[1], 1)],
            ],
        )
        proj_chain(wq_sb, qt_sb, xt_sb, 2, 0)
        proj_chain(wq_sb, qt_sb, xt_sb, 2, 1)
        proj_chain(wk_sb, kt_sb, yt_sb, 2, 0)

        # pair 2: KT2-ic1 first (needed by ST jt4), ctx of heads 2/3, QT3/KT3
        pt2 = st_pair(
            2,
            [
                [mk(proj_chain, wk_sb, kt_sb, yt_sb, 2, 1)],
                [mk(ctx_chain, 2, pt1[0], 0)],
                [mk(ctx_chain, 2, pt1[0], 1)],
                [mk(ctx_chain, 3, pt1[1], 0)],
                [mk(ctx_chain, 3, pt1[1], 1)],
                [mk(proj_chain, wq_sb, qt_sb, xt_sb, 3, 0)],
                [mk(proj_chain, wq_sb, qt_sb, xt_sb, 3, 1)],
                [mk(proj_chain, wk_sb, kt_sb, yt_sb, 3, 0)],
            ],
        )
        proj_chain(wk_sb, kt_sb, yt_sb, 3, 1)

        # pair 3: ctx of heads 4/5, then out-proj partial A over dt 0..2
        # (heads 0..5). poA[8:16] stays after the st loop to cover the
        # final exp drain that the tail ctx chains must wait for.
        poA = [
            mk(po_chain, it, oc, (0, 1, 2), out_a)
            for it in range(IT)
            for oc in range(NIC)
        ]
        pt3 = st_pair(
            3,
            [
                [mk(ctx_chain, 4, pt2[0], 0)],
                [mk(ctx_chain, 4, pt2[0], 1)],
                [mk(ctx_chain, 5, pt2[1], 0)],
                [mk(ctx_chain, 5, pt2[1], 1)],
                poA[0:3],
                poA[3:6],
                poA[6:9],
                poA[9:11],
            ],
        )
        for f in poA[11:16]:
            f()
        flush_deferred()

        # tail: the three ctx chains whose normalize round-trips can hide
        # under later work come first (60/70/71), out blocks it0/1 start as
        # soon as fin60/fin70 flush, ctx61 and the rest pipeline behind.
        # dt3 out-projection uses 2-bank-wide chains with drains
        # alternating scalar/vector and output DMAs split across 2 queues.
        def owide(it):
            po_wide_chain(
                it, (3,), out_c,
                "scalar" if it % 2 == 0 else "vector",
                nc.sync.dma_start if it % 2 == 0 else nc.gpsimd.dma_start,
            )

        ctx_chain(6, pt3[0], 0)          # queues fin60
        pending = list(deferred)
        deferred.clear()
        ctx_chain(7, pt3[1], 0)          # queues fin70
        for f in pending:                # fin60
            f()
        pending = list(deferred)
        deferred.clear()
        ctx_chain(7, pt3[1], 1, fast_norm=True)   # queues fin71
        for f in pending:                # fin70
            f()
        owide(0)                         # needs fin60/fin70 only
        owide(1)
        pending = list(deferred)
        deferred.clear()
        ctx_chain(6, pt3[0], 1, fast_norm=True)   # queues fin61
        for f in pending:                # fin71
            f()
        owide(2)
        owide(3)
        flush_deferred()                 # fin61
        for it in range(IT // 2, IT):
            owide(it)


_NC_CACHE = None


def _get_nc():
    global _NC_CACHE
    if _NC_CACHE is None:
        _NC_CACHE = _build_kernel()
    return _NC_CACHE


def kernel(x, y, Wq, Wk, Wv, Wo, _trace=False):
    bf = ml_dtypes.bfloat16
    f8 = ml_dtypes.float8_e4m3
    x = np.asarray(x, np.float32)
    y = np.asarray(y, np.float32)
    xtb = [np.ascontiguousarray(np.asarray(x[b]).T).astype(bf) for b in range(B)]
    ytb = [np.ascontiguousarray(np.asarray(y[b]).T).astype(bf) for b in range(B)]
    wqs = [np.ascontiguousarray(np.asarray(Wq)[:, t * DL : (t + 1) * DL]).astype(bf) for t in range(TP)]
    wks = [np.ascontiguousarray(np.asarray(Wk)[:, t * DL : (t + 1) * DL]).astype(bf) for t in range(TP)]
    wvs = [np.ascontiguousarray(np.asarray(Wv)[:, t * DL : (t + 1) * DL]).astype(bf) for t in range(TP)]
    wos = [np.ascontiguousarray(np.asarray(Wo)[t * DL : (t + 1) * DL, :]).astype(bf) for t in range(TP)]

    in_maps = []
    for b in range(B):
        for t in range(TP):
            in_maps.append(
                {
                    "xt": xtb[b],
                    "yt": ytb[b],
                    "wq": wqs[t],
                    "wk": wks[t],
                    "wv": wvs[t],
                    "wo": wos[t],
                }
            )

    nc = _get_nc()
    res = run_bass_kernel_spmd(
        nc, in_maps, core_ids=list(range(N_CORES)), trace=_trace
    )
    out = np.empty((B, L, U), np.float32)
    for b in range(B):
        out[b] = (
            np.asarray(res.results[2 * b]["out_a"], np.float32)
            + np.asarray(res.results[2 * b]["out_c"], np.float32)
            + np.asarray(res.results[2 * b + 1]["out_a"], np.float32)
            + np.asarray(res.results[2 * b + 1]["out_c"], np.float32)
        )
    if _trace:
        return out, res
    return out


# revision 29
# speedup vs baseline: 1.0495x; 1.0142x over previous
"""Multi-head attention (B=4, L=1024, D=1024, H=16, DH=64) on 8 TRN2 NeuronCores.

Sharding: data-parallel over batch (4) x tensor-parallel over heads (2).
Core c = 2*b + t computes, for batch b, heads [t*8, (t+1)*8):
    QT = Wq_t^T X^T, KT = Wk_t^T X^T, V = Y Wv_t        (all bf16 matmuls)
    per head: S^T = K_h Q_h^T; P^T = exp(S^T/8);
              [ctx^T; rowsum] = Vaug_h^T P^T;  ctxn = ctx / rowsum
    O_partial = ctxn^T Wo_t                              (f32, two dt-halves)
Host pre-transposes X/Y, casts to bf16, and sums the four f32 partials
(2 tensor-parallel cores x 2 dt-halves) per batch.

Schedule notes (engines execute their streams in order, so emission order is
the software pipeline):
  - Input DMAs are spread across 4 engine queues (sync/vector/gpsimd/scalar)
    with the critical prefix (wq/wk dt0 columns, xt, yt ic0-half) issued
    first on each queue; issuing everything from one queue serializes at
    ~0.6us per descriptor-gen instruction.
  - ~20 dummy matmuls run during the unavoidable input-DMA wait so the PE
    HAM clock-gate warms up (K=8/8) before real matmuls start; otherwise the
    first ~20us of matmuls run at 1.2GHz instead of 2.4GHz.
  - Score matmuls for the head pair are emitted interleaved (h0,h1,h0,h1);
    the two heads' K blocks sit at partitions 0:64 / 64:128 so the MMs land
    in different PE row groups and run concurrently (~2x on the ST phase).
  - Every ST (scores) step, whose exp drain on the scalar engine is slower
    than the matmuls, is followed by an independent fill chain (V projection,
    next d-tile QT/KT, an earlier head's ctx, or an out-projection partial)
    so the tensor engine never waits for the scalar engine.
  - out_a (Wo over dt 0..2) drains during the last score phase; out_c (dt3)
    uses 2-bank-wide PSUM chains at the tail with drains alternating between
    the scalar and vector engines and output DMAs split over two queues.
"""

import numpy as np
import ml_dtypes

import concourse.tile as tile
import concourse.mybir as mybir
from concourse import bacc
from concourse.bass_utils import run_bass_kernel_spmd

B, L, D, U, H = 4, 1024, 1024, 1024, 16
DH = U // H          # 64 head dim
TP = 2               # tensor-parallel ways (heads)
DL = U // TP         # 512 local units
HL = H // TP         # 8 local heads
P = 128              # partitions
NI = 512             # matmul free-dim chunk (one PSUM bank of f32)
CC = D // P          # 8 contraction chunks for projections
DT = DL // P         # 4 local d-tiles
IT = L // P          # 8 i/j tiles
NIC = L // NI        # 2 free chunks of 512
N_CORES = 8
N_DUMMY = 35         # HAM warm-up matmuls during the input-DMA wait

BF16 = mybir.dt.bfloat16
F32 = mybir.dt.float32
F8 = mybir.dt.float8e4


def _build_kernel():
    nc = bacc.Bacc(
        "TRN2", target_bir_lowering=False, debug=False, num_devices=N_CORES
    )
    xt = nc.dram_tensor("xt", [D, L], BF16, kind="ExternalInput").ap()
    yt = nc.dram_tensor("yt", [D, L], BF16, kind="ExternalInput").ap()
    wq = nc.dram_tensor("wq", [D, DL], BF16, kind="ExternalInput").ap()
    wk = nc.dram_tensor("wk", [D, DL], BF16, kind="ExternalInput").ap()
    wv = nc.dram_tensor("wv", [D, DL], BF16, kind="ExternalInput").ap()
    wo = nc.dram_tensor("wo", [DL, U], BF16, kind="ExternalInput").ap()
    out_a = nc.dram_tensor("out_a", [L, U], BF16, kind="ExternalOutput").ap()
    out_c = nc.dram_tensor("out_c", [L, U], BF16, kind="ExternalOutput").ap()

    with tile.TileContext(nc) as tc:
        _mha_body(tc, out_a, out_c, xt, yt, wq, wk, wv, wo)

    nc.compile()
    return nc


def _mha_body(tc, out_a, out_c, xt, yt, wq, wk, wv, wo):
    nc = tc.nc
    from contextlib import ExitStack

    with ExitStack() as ctx:
        persist = ctx.enter_context(tc.tile_pool(name="persist", bufs=1))
        pt_pool = ctx.enter_context(tc.tile_pool(name="pt", bufs=4))
        # ST tiles: [P, 1024] f32 = 2 banks each
        ps_wide = ctx.enter_context(tc.tile_pool(name="ps_wide", bufs=2, space="PSUM"))
        # single-bank accumulators (projections, V, ctx, out-proj)
        ps_acc = ctx.enter_context(tc.tile_pool(name="ps_acc", bufs=4, space="PSUM"))
        small = ctx.enter_context(tc.tile_pool(name="small", bufs=4))

        # persistent SBUF tensors (activations/QKV weights in fp8: the
        # projections run DoubleRow with 256-deep contraction per step)
        xt_sb = persist.tile([P, CC, L], BF16, tag="xt")
        yt_sb = persist.tile([P, CC, L], BF16, tag="yt")
        wq_sb = persist.tile([P, CC, DL], BF16, tag="wq")
        wk_sb = persist.tile([P, CC, DL], BF16, tag="wk")
        wv_sb = persist.tile([P, CC, DL], BF16, tag="wv")
        wo_sb = persist.tile([P, DT, U], BF16, tag="wo")
        qt_sb = persist.tile([P, DT, L], BF16, tag="qt")
        kt_sb = persist.tile([P, DT, L], BF16, tag="kt")
        # Vaug: per j-chunk, per head a 128-col block; even h: [V_h | ones],
        # odd h: [ones | V_h] (ctx^T lands on the head's own cx partitions)
        va_sb = persist.tile([P, IT, HL * P], BF16, tag="va")
        cx_sb = persist.tile([P, DT, L], BF16, tag="cx")
        dummy_sb = persist.tile([P, NI], BF16, tag="dummy")

        wq_r = wq.rearrange("(cc p) d -> p cc d", p=P)
        wk_r = wk.rearrange("(cc p) d -> p cc d", p=P)
        wv_r = wv.rearrange("(cc p) d -> p cc d", p=P)
        xt_r = xt.rearrange("(cc p) i -> p cc i", p=P)
        yt_r = yt.rearrange("(cc p) i -> p cc i", p=P)
        wo_r = wo.rearrange("(dt p) o -> p dt o", p=P)

        # ---- input DMAs: 3 hw queues (SP/gpsimd/Activation). Class-1
        # (wq/wk dt0 cols, xt, yt ic0 — gates the first score phase) is
        # split evenly across the queue heads; class-2 strictly behind.
        nc.vector.memset(dummy_sb[:], 1.0)
        # class1 gates the first score phase; class2 (yt-ic1, wv) gates the
        # pair-0 fill chains; class3 (wq/wk rest) gates pair-1; wo is last.
        # sync: wq-dt0, xt 0..2, yt-ic0 0..1 | yt-ic1 0..3, wv 0..1 | ...
        nc.sync.dma_start(out=wq_sb[:, :, 0:P], in_=wq_r[:, :, 0:P])
        for cc in range(3):
            nc.sync.dma_start(out=xt_sb[:, cc], in_=xt_r[:, cc])
        for cc in range(2):
            nc.sync.dma_start(out=yt_sb[:, cc, 0:NI], in_=yt_r[:, cc, 0:NI])
        for cc in range(4):
            nc.sync.dma_start(out=yt_sb[:, cc, NI:L], in_=yt_r[:, cc, NI:L])
        for cc in range(2):
            nc.sync.dma_start(out=wv_sb[:, cc], in_=wv_r[:, cc])
        nc.sync.dma_start(out=wq_sb[:, :, P:DL], in_=wq_r[:, :, P:DL])
        for dt in range(2):
            nc.sync.dma_start(out=wo_sb[:, dt], in_=wo_r[:, dt])
        # gpsimd: wk-dt0, xt 3..5, yt-ic0 2..3 | yt-ic1 4..7, wv 2..3 | ...
        nc.gpsimd.dma_start(out=wk_sb[:, :, 0:P], in_=wk_r[:, :, 0:P])
        for cc in range(3, 6):
            nc.gpsimd.dma_start(out=xt_sb[:, cc], in_=xt_r[:, cc])
        for cc in range(2, 4):
            nc.gpsimd.dma_start(out=yt_sb[:, cc, 0:NI], in_=yt_r[:, cc, 0:NI])
        for cc in range(4, 8):
            nc.gpsimd.dma_start(out=yt_sb[:, cc, NI:L], in_=yt_r[:, cc, NI:L])
        for cc in range(2, 4):
            nc.gpsimd.dma_start(out=wv_sb[:, cc], in_=wv_r[:, cc])
        nc.gpsimd.dma_start(out=wk_sb[:, :, P:DL], in_=wk_r[:, :, P:DL])
        for dt in range(2, 4):
            nc.gpsimd.dma_start(out=wo_sb[:, dt], in_=wo_r[:, dt])
        # scalar: xt 6..7, yt-ic0 4..7 | wv 4..7
        for cc in range(6, 8):
            nc.scalar.dma_start(out=xt_sb[:, cc], in_=xt_r[:, cc])
        for cc in range(4, 8):
            nc.scalar.dma_start(out=yt_sb[:, cc, 0:NI], in_=yt_r[:, cc, 0:NI])
        for cc in range(4, 8):
            nc.scalar.dma_start(out=wv_sb[:, cc], in_=wv_r[:, cc])

        # ones blocks of Vaug only (even h: cols DH:P, odd h: cols 0:DH)
        va_r = va_sb.rearrange("p it (h s) -> p it h s", s=P)
        nc.vector.memset(va_r[:, :, 0::2, DH:P], 1.0)
        nc.gpsimd.memset(va_r[:, :, 1::2, 0:DH], 1.0)
        # f32 identity for the tail partition-move matmuls
        from concourse.masks import make_identity

        id_sb = persist.tile([P, P], F32, tag="ident")
        make_identity(nc, id_sb)

        # ---- HAM warm-up: dummy matmuls while input DMA streams in ----
        ps_dummy = ps_wide.tile([P, 2 * NI], F32, tag="wide")
        for _ in range(N_DUMMY):
            nc.tensor.matmul(
                ps_dummy[:, 0:NI], dummy_sb[:, 0:P], dummy_sb[:], start=True, stop=True
            )

        scale = DH**-0.5

        # ---- chain emitters (each a short burst of independent PE work) ----

        DR = mybir.MatmulPerfMode.DoubleRow

        def proj_chain(w_sb, t_sb, rhs_sb, dt, ic):
            ps = ps_acc.tile([P, NI], F32, tag="acc")
            for cc in range(CC):
                nc.tensor.matmul(
                    ps[:],
                    w_sb[:, cc, dt * P : (dt + 1) * P],
                    rhs_sb[:, cc, ic * NI : (ic + 1) * NI],
                    start=(cc == 0),
                    stop=(cc == CC - 1),
                )
            nc.vector.tensor_copy(t_sb[:, dt, ic * NI : (ic + 1) * NI], ps[:])

        def proj_chain2(w_sb, t_sb, rhs_sb, dt):
            # both ic halves in one emission: consecutive matmuls share the
            # stationary operand (bass dedupes the LDWEIGHTS) and there's a
            # single chain-boundary weight-load exposure instead of two
            ps0 = ps_acc.tile([P, NI], F32, tag="acc")
            ps1 = ps_acc.tile([P, NI], F32, tag="acc")
            for cc in range(CC):
                for ic, ps in ((0, ps0), (1, ps1)):
                    nc.tensor.matmul(
                        ps[:],
                        w_sb[:, cc, dt * P : (dt + 1) * P],
                        rhs_sb[:, cc, ic * NI : (ic + 1) * NI],
                        start=(cc == 0),
                        stop=(cc == CC - 1),
                    )
            nc.vector.tensor_copy(t_sb[:, dt, 0:NI], ps0[:])
            nc.vector.tensor_copy(t_sb[:, dt, NI:L], ps1[:])

        def v_chain(jt):
            ps = ps_acc.tile([P, NI], F32, tag="acc")
            for cc in range(CC):
                nc.tensor.matmul(
                    ps[:],
                    yt_sb[:, cc, jt * P : (jt + 1) * P],
                    wv_sb[:, cc, :],
                    start=(cc == 0),
                    stop=(cc == CC - 1),
                )
            va_blk = va_sb[:, jt].rearrange("p (h s) -> p h s", s=P)
            ps_blk = ps.rearrange("p (h s) -> p h s", s=DH)
            nc.vector.tensor_copy(va_blk[:, 0::2, 0:DH], ps_blk[:, 0::2, :])
            nc.vector.tensor_copy(va_blk[:, 1::2, DH:P], ps_blk[:, 1::2, :])

        # Deferred finishers: the normalize needs a SBUF->SBUF DMA between
        # two DVE ops; emitting the post-DMA ops immediately would stall the
        # in-order DVE stream (and the PSUM-releasing copies queued behind
        # it) for the DMA round-trip. Instead each ctx chain queues its
        # post-DMA ops and the next fill slot flushes them.
        deferred = []

        def flush_deferred():
            while deferred:
                deferred.pop(0)()

        def ctx_chain(h, ptile, ic, fast_norm=False):
            dt, r0 = divmod(h * DH, P)
            ct = ps_acc.tile([P, NI], F32, tag="acc")
            for jt in range(IT):
                nc.tensor.matmul(
                    ct[:],
                    va_sb[:, jt, h * P : (h + 1) * P],
                    ptile[:, jt, ic * NI : (ic + 1) * NI],
                    start=(jt == 0),
                    stop=(jt == IT - 1),
                )
            # custom DVE ops (reciprocal) only work at base partition 0, and
            # ctx/rowsum land on complementary partition halves. Mid-kernel
            # the move goes through a SBUF->SBUF DMA whose round-trip hides
            # under later fill slots; at the tail (fast_norm) a tiny
            # identity matmul does the partition move with ~1us less
            # latency at the cost of a PE slot and a PSUM bank.
            rc = small.tile([P, NI], F32, tag="rc")
            if r0 == 0:
                rs = small.tile([P, NI], F32, tag="rs")
                nc.vector.tensor_copy(rs[DH:P, :], ct[DH:P, :])
                if fast_norm:
                    pr = ps_acc.tile([P, NI], F32, tag="acc")
                    nc.tensor.matmul(
                        pr[0:DH, :], id_sb[DH:P, DH:P], rs[DH:P, :],
                        start=True, stop=True,
                    )

                    def fin():
                        nc.vector.reciprocal_approx_fast(rc[0:DH, :], pr[0:DH, :])
                        nc.vector.tensor_mul(
                            cx_sb[0:DH, dt, ic * NI : (ic + 1) * NI],
                            ct[0:DH, :],
                            rc[0:DH, :],
                        )
                else:
                    nc.gpsimd.dma_start(out=rs[0:DH, :], in_=rs[DH:P, :])

                    def fin():
                        nc.vector.reciprocal_approx_fast(rc[0:DH, :], rs[0:DH, :])
                        nc.vector.tensor_mul(
                            cx_sb[0:DH, dt, ic * NI : (ic + 1) * NI],
                            ct[0:DH, :],
                            rc[0:DH, :],
                        )
            else:
                nc.vector.reciprocal_approx_fast(rc[0:DH, :], ct[0:DH, :])
                if fast_norm:
                    pr = ps_acc.tile([P, NI], F32, tag="acc")
                    nc.tensor.matmul(
                        pr[DH:P, :], id_sb[0:DH, 0:DH], rc[0:DH, :],
                        start=True, stop=True,
                    )
                    # DVE can read only one PSUM operand per instruction
                    rc2 = small.tile([P, NI], F32, tag="rc2")
                    nc.vector.tensor_copy(rc2[DH:P, :], pr[DH:P, :])

                    def fin():
                        nc.vector.tensor_mul(
                            cx_sb[DH:P, dt, ic * NI : (ic + 1) * NI],
                            ct[DH:P, :],
                            rc2[DH:P, :],
                        )
                else:
                    nc.gpsimd.dma_start(out=rc[DH:P, :], in_=rc[0:DH, :])

                    def fin():
                        nc.vector.tensor_mul(
                            cx_sb[DH:P, dt, ic * NI : (ic + 1) * NI],
                            ct[DH:P, :],
                            rc[DH:P, :],
                        )

            deferred.append(fin)

        def po2_chain(it, dts, out_ap):
            # dt-major, oc-interleaved: consecutive matmuls share the
            # stationary cx block so the weight load is deduped; one
            # 256KB DMA for both oc halves
            po0 = ps_acc.tile([P, NI], F32, tag="acc")
            po1 = ps_acc.tile([P, NI], F32, tag="acc")
            for k, dt in enumerate(dts):
                for oc, po in ((0, po0), (1, po1)):
                    nc.tensor.matmul(
                        po[:],
                        cx_sb[:, dt, it * P : (it + 1) * P],
                        wo_sb[:, dt, oc * NI : (oc + 1) * NI],
                        start=(k == 0),
                        stop=(k == len(dts) - 1),
                    )
            o_st = small.tile([P, 2 * NI], BF16, tag="ostw")
            nc.vector.tensor_copy(o_st[:, 0:NI], po0[:])
            nc.vector.tensor_copy(o_st[:, NI:L], po1[:])
            out_r = out_ap.rearrange("(it p) o -> it p o", p=P)
            nc.sync.dma_start(out=out_r[it], in_=o_st[:])

        def po_chain(it, oc, dts, out_ap):
            # out-projection partial over the given d-tiles
            po = ps_acc.tile([P, NI], F32, tag="acc")
            for k, dt in enumerate(dts):
                nc.tensor.matmul(
                    po[:],
                    cx_sb[:, dt, it * P : (it + 1) * P],
                    wo_sb[:, dt, oc * NI : (oc + 1) * NI],
                    start=(k == 0),
                    stop=(k == len(dts) - 1),
                )
            o_st = small.tile([P, NI], BF16, tag="ost")
            nc.vector.tensor_copy(o_st[:], po[:])
            out_r = out_ap.rearrange("(it p) o -> it p o", p=P)
            nc.sync.dma_start(
                out=out_r[it, :, oc * NI : (oc + 1) * NI], in_=o_st[:]
            )

        def po_wide_chain(it, dts, out_ap, copy_eng, dma_eng):
            # both oc halves in one 2-bank psum tile (the ST pool slots are
            # idle once the score phases are done); single FD=1024 bf16
            # drain + one 256KB DMA
            po = ps_wide.tile([P, 2 * NI], F32, tag="wide")
            for oc in range(NIC):
                for k, dt in enumerate(dts):
                    nc.tensor.matmul(
                        po[:, oc * NI : (oc + 1) * NI],
                        cx_sb[:, dt, it * P : (it + 1) * P],
                        wo_sb[:, dt, oc * NI : (oc + 1) * NI],
                        start=(k == 0),
                        stop=(k == len(dts) - 1),
                    )
            o_st = small.tile([P, 2 * NI], BF16, tag="ostw")
            if copy_eng == "scalar":
                nc.scalar.copy(o_st[:], po[:])
            else:
                nc.vector.tensor_copy(o_st[:], po[:])
            out_r = out_ap.rearrange("(it p) o -> it p o", p=P)
            dma_eng(out=out_r[it], in_=o_st[:])

        # ---- ST + exp for a head pair, fill chains between steps ----
        # The two heads' score MMs are interleaved so they run in different
        # PE row groups (partitions 0:64 vs 64:128) concurrently.

        def st_pair(hp, fills):
            dt = hp
            ptiles = []
            for h_off in range(2):
                pt_tile = pt_pool.tile([P, IT, L], BF16, tag="pt")
                ptiles.append(pt_tile)
            fills = list(fills)
            for jt in range(IT):
                sts = [
                    ps_wide.tile([P, 2 * NI], F32, tag="wide", name=f"st{h_off}")
                    for h_off in range(2)
                ]
                # h-outer/ic-inner: bass dedupes the LDWEIGHTS across the
                # two ic matmuls of a head, and h1's weight load pulls
                # ahead into its own row group during h0's matmuls, so
                # h1-ic0 runs concurrently with h0-ic1 (3 slots per step).
                # Alternating h0/h1 would force a weight reload per MM
                # that serializes against the in-flight MM's row group.
                for h_off in range(2):
                    r0 = DH * h_off
                    for ic in range(NIC):
                        nc.tensor.matmul(
                            sts[h_off][:, ic * NI : (ic + 1) * NI],
                            kt_sb[r0 : r0 + DH, dt, jt * P : (jt + 1) * P],
                            qt_sb[r0 : r0 + DH, dt, ic * NI : (ic + 1) * NI],
                            start=True,
                            stop=True,
                        )
                for h_off in range(2):
                    nc.scalar.activation(
                        ptiles[h_off][:, jt, :],
                        sts[h_off][:],
                        mybir.ActivationFunctionType.Exp,
                        scale=scale,
                    )
                if jt < len(fills):
                    pending = list(deferred)
                    deferred.clear()
                    for f in fills[jt]:
                        f()
                    for f in pending:
                        f()
            return ptiles

        # ---- schedule ----
        mk = lambda f, *a: (lambda: f(*a))

        # dt0 projections gate the first score phase; only these run before
        # pair 0 (everything else becomes fill work). K's ic1 half stays a
        # fill: it needs yt-ic1 (class-2 DMA) and is only read by ST jt4.
        proj_chain2(wq_sb, qt_sb, xt_sb, 0)
        proj_chain(wk_sb, kt_sb, yt_sb, 0, 0)

        # pair 0: fill with K-dt0-ic1, V chains and QT1/KT1
        pt0 = st_pair(
            0,
            [
                [mk(proj_chain, wk_sb, kt_sb, yt_sb, 0, 1)],
                [mk(v_chain, 0)],
                [mk(v_chain, 1)],
                [mk(v_chain, 2)],
                [mk(v_chain, 3)],
                [mk(proj_chain2, wq_sb, qt_sb, xt_sb, 1)],
                [mk(proj_chain, wk_sb, kt_sb, yt_sb, 1, 0)],
            ],
        )
        # KT1-ic1 covers pair-0's exp drain (needed by pair-1 ST jt4)
        proj_chain(wk_sb, kt_sb, yt_sb, 1, 1)

        # pair 1: remaining V chains first (ctx h0 reads all of va), then
        # ctx of heads 0/1 and QT2
        pt1 = st_pair(
            1,
            [
                [mk(v_chain, 4)],
                [mk(v_chain, 5)],
                [mk(v_chain, 6)],
                [mk(v_chain, 7)],
                [mk(ctx_chain, 0, pt0[0], 0)],
                [mk(ctx_chain, 0, pt0[0], 1)],
                [mk(ctx_chain, 1, pt0[1], 0)],
                [mk(ctx_chain, 1, pt0[1], 1)],
            ],
        )
        proj_chain2(wq_sb, qt_sb, xt_sb, 2)
        proj_chain(wk_sb, kt_sb, yt_sb, 2, 0)

        # pair 2: KT2-ic1 first (needed by ST jt4), ctx of heads 2/3, QT3/KT3
        pt2 = st_pair(
            2,
            [
                [mk(proj_chain, wk_sb, kt_sb, yt_sb, 2, 1)],
                [mk(ctx_chain, 2, pt1[0], 0)],
                [mk(ctx_chain, 2, pt1[0], 1)],
                [mk(ctx_chain, 3, pt1[1], 0)],
                [mk(ctx_chain, 3, pt1[1], 1)],
                [mk(proj_chain2, wq_sb, qt_sb, xt_sb, 3)],
                [mk(proj_chain, wk_sb, kt_sb, yt_sb, 3, 0)],
            ],
        )
        proj_chain(wk_sb, kt_sb, yt_sb, 3, 1)

        # pair 3: ctx of heads 4/5, then out-proj partial A over dt 0..2
        # (heads 0..5). poA[8:16] stays after the st loop to cover the
        # final exp drain that the tail ctx chains must wait for.
        poA = [
            mk(po2_chain, it, (0, 1, 2), out_a) for it in range(IT)
        ]
        pt3 = st_pair(
            3,
            [
                [mk(ctx_chain, 4, pt2[0], 0)],
                [mk(ctx_chain, 4, pt2[0], 1)],
                [mk(ctx_chain, 5, pt2[1], 0)],
                [mk(ctx_chain, 5, pt2[1], 1)],
                poA[0:1],
                poA[1:2],
                poA[2:4],
                poA[4:6],
            ],
        )
        for f in poA[6:8]:
            f()
        flush_deferred()

        # tail: the three ctx chains whose normalize round-trips can hide
        # under later work come first (60/70/71), out blocks it0/1 start as
        # soon as fin60/fin70 flush, ctx61 and the rest pipeline behind.
        # dt3 out-projection uses 2-bank-wide chains with drains
        # alternating scalar/vector and output DMAs split across 2 queues.
        def owide(it):
            po_wide_chain(
                it, (3,), out_c,
                "scalar" if it % 2 == 0 else "vector",
                nc.sync.dma_start if it % 2 == 0 else nc.gpsimd.dma_start,
            )

        ctx_chain(6, pt3[0], 0)          # queues fin60
        pending = list(deferred)
        deferred.clear()
        ctx_chain(7, pt3[1], 0)          # queues fin70
        for f in pending:                # fin60
            f()
        pending = list(deferred)
        deferred.clear()
        ctx_chain(7, pt3[1], 1, fast_norm=True)   # queues fin71
        for f in pending:                # fin70
            f()
        owide(0)                         # needs fin60/fin70 only
        owide(1)
        pending = list(deferred)
        deferred.clear()
        ctx_chain(6, pt3[0], 1, fast_norm=True)   # queues fin61
        for f in pending:                # fin71
            f()
        owide(2)
        owide(3)
        flush_deferred()                 # fin61
        for it in range(IT // 2, IT):
            owide(it)


_NC_CACHE = None


def _get_nc():
    global _NC_CACHE
    if _NC_CACHE is None:
        _NC_CACHE = _build_kernel()
    return _NC_CACHE


def kernel(x, y, Wq, Wk, Wv, Wo, _trace=False):
    bf = ml_dtypes.bfloat16
    f8 = ml_dtypes.float8_e4m3
    x = np.asarray(x, np.float32)
    y = np.asarray(y, np.float32)
    xtb = [np.ascontiguousarray(np.asarray(x[b]).T).astype(bf) for b in range(B)]
    ytb = [np.ascontiguousarray(np.asarray(y[b]).T).astype(bf) for b in range(B)]
    wqs = [np.ascontiguousarray(np.asarray(Wq)[:, t * DL : (t + 1) * DL]).astype(bf) for t in range(TP)]
    wks = [np.ascontiguousarray(np.asarray(Wk)[:, t * DL : (t + 1) * DL]).astype(bf) for t in range(TP)]
    wvs = [np.ascontiguousarray(np.asarray(Wv)[:, t * DL : (t + 1) * DL]).astype(bf) for t in range(TP)]
    wos = [np.ascontiguousarray(np.asarray(Wo)[t * DL : (t + 1) * DL, :]).astype(bf) for t in range(TP)]

    in_maps = []
    for b in range(B):
        for t in range(TP):
            in_maps.append(
                {
                    "xt": xtb[b],
                    "yt": ytb[b],
                    "wq": wqs[t],
                    "wk": wks[t],
                    "wv": wvs[t],
                    "wo": wos[t],
                }
            )

    nc = _get_nc()
    res = run_bass_kernel_spmd(
        nc, in_maps, core_ids=list(range(N_CORES)), trace=_trace
    )
    out = np.empty((B, L, U), np.float32)
    for b in range(B):
        out[b] = (
            np.asarray(res.results[2 * b]["out_a"], np.float32)
            + np.asarray(res.results[2 * b]["out_c"], np.float32)
            + np.asarray(res.results[2 * b + 1]["out_a"], np.float32)
            + np.asarray(res.results[2 * b + 1]["out_c"], np.float32)
        )
    if _trace:
        return out, res
    return out


# revision 30
# speedup vs baseline: 1.0499x; 1.0003x over previous
"""Multi-head attention (B=4, L=1024, D=1024, H=16, DH=64) on 8 TRN2 NeuronCores.

Sharding: data-parallel over batch (4) x tensor-parallel over heads (2).
Core c = 2*b + t computes, for batch b, heads [t*8, (t+1)*8):
    QT = Wq_t^T X^T, KT = Wk_t^T X^T, V = Y Wv_t        (all bf16 matmuls)
    per head: S^T = K_h Q_h^T; P^T = exp(S^T/8);
              [ctx^T; rowsum] = Vaug_h^T P^T;  ctxn = ctx / rowsum
    O_partial = ctxn^T Wo_t                              (f32, two dt-halves)
Host pre-transposes X/Y, casts to bf16, and sums the four f32 partials
(2 tensor-parallel cores x 2 dt-halves) per batch.

Schedule notes (engines execute their streams in order, so emission order is
the software pipeline):
  - Input DMAs are spread across 4 engine queues (sync/vector/gpsimd/scalar)
    with the critical prefix (wq/wk dt0 columns, xt, yt ic0-half) issued
    first on each queue; issuing everything from one queue serializes at
    ~0.6us per descriptor-gen instruction.
  - ~20 dummy matmuls run during the unavoidable input-DMA wait so the PE
    HAM clock-gate warms up (K=8/8) before real matmuls start; otherwise the
    first ~20us of matmuls run at 1.2GHz instead of 2.4GHz.
  - Score matmuls for the head pair are emitted interleaved (h0,h1,h0,h1);
    the two heads' K blocks sit at partitions 0:64 / 64:128 so the MMs land
    in different PE row groups and run concurrently (~2x on the ST phase).
  - Every ST (scores) step, whose exp drain on the scalar engine is slower
    than the matmuls, is followed by an independent fill chain (V projection,
    next d-tile QT/KT, an earlier head's ctx, or an out-projection partial)
    so the tensor engine never waits for the scalar engine.
  - out_a (Wo over dt 0..2) drains during the last score phase; out_c (dt3)
    uses 2-bank-wide PSUM chains at the tail with drains alternating between
    the scalar and vector engines and output DMAs split over two queues.
"""

import numpy as np
import ml_dtypes

import concourse.tile as tile
import concourse.mybir as mybir
from concourse import bacc
from concourse.bass_utils import run_bass_kernel_spmd

B, L, D, U, H = 4, 1024, 1024, 1024, 16
DH = U // H          # 64 head dim
TP = 2               # tensor-parallel ways (heads)
DL = U // TP         # 512 local units
HL = H // TP         # 8 local heads
P = 128              # partitions
NI = 512             # matmul free-dim chunk (one PSUM bank of f32)
CC = D // P          # 8 contraction chunks for projections
DT = DL // P         # 4 local d-tiles
IT = L // P          # 8 i/j tiles
NIC = L // NI        # 2 free chunks of 512
N_CORES = 8
N_DUMMY = 35         # HAM warm-up matmuls during the input-DMA wait

BF16 = mybir.dt.bfloat16
F32 = mybir.dt.float32
F8 = mybir.dt.float8e4


def _build_kernel():
    nc = bacc.Bacc(
        "TRN2", target_bir_lowering=False, debug=False, num_devices=N_CORES
    )
    xt = nc.dram_tensor("xt", [D, L], BF16, kind="ExternalInput").ap()
    yt = nc.dram_tensor("yt", [D, L], BF16, kind="ExternalInput").ap()
    wq = nc.dram_tensor("wq", [D, DL], BF16, kind="ExternalInput").ap()
    wk = nc.dram_tensor("wk", [D, DL], BF16, kind="ExternalInput").ap()
    wv = nc.dram_tensor("wv", [D, DL], BF16, kind="ExternalInput").ap()
    wo = nc.dram_tensor("wo", [DL, U], BF16, kind="ExternalInput").ap()
    out_a = nc.dram_tensor("out_a", [L, U], BF16, kind="ExternalOutput").ap()
    out_c = nc.dram_tensor("out_c", [L, U], BF16, kind="ExternalOutput").ap()

    with tile.TileContext(nc) as tc:
        _mha_body(tc, out_a, out_c, xt, yt, wq, wk, wv, wo)

    nc.compile()
    return nc


def _mha_body(tc, out_a, out_c, xt, yt, wq, wk, wv, wo):
    nc = tc.nc
    from contextlib import ExitStack

    with ExitStack() as ctx:
        persist = ctx.enter_context(tc.tile_pool(name="persist", bufs=1))
        pt_pool = ctx.enter_context(tc.tile_pool(name="pt", bufs=4))
        # ST tiles: [P, 1024] f32 = 2 banks each
        ps_wide = ctx.enter_context(tc.tile_pool(name="ps_wide", bufs=2, space="PSUM"))
        # single-bank accumulators (projections, V, ctx, out-proj)
        ps_acc = ctx.enter_context(tc.tile_pool(name="ps_acc", bufs=4, space="PSUM"))
        small = ctx.enter_context(tc.tile_pool(name="small", bufs=4))

        # persistent SBUF tensors (activations/QKV weights in fp8: the
        # projections run DoubleRow with 256-deep contraction per step)
        xt_sb = persist.tile([P, CC, L], BF16, tag="xt")
        yt_sb = persist.tile([P, CC, L], BF16, tag="yt")
        wq_sb = persist.tile([P, CC, DL], BF16, tag="wq")
        wk_sb = persist.tile([P, CC, DL], BF16, tag="wk")
        wv_sb = persist.tile([P, CC, DL], BF16, tag="wv")
        wo_sb = persist.tile([P, DT, U], BF16, tag="wo")
        qt_sb = persist.tile([P, DT, L], BF16, tag="qt")
        kt_sb = persist.tile([P, DT, L], BF16, tag="kt")
        # Vaug: per j-chunk, per head a 128-col block; even h: [V_h | ones],
        # odd h: [ones | V_h] (ctx^T lands on the head's own cx partitions)
        va_sb = persist.tile([P, IT, HL * P], BF16, tag="va")
        cx_sb = persist.tile([P, DT, L], BF16, tag="cx")
        dummy_sb = persist.tile([P, NI], BF16, tag="dummy")

        wq_r = wq.rearrange("(cc p) d -> p cc d", p=P)
        wk_r = wk.rearrange("(cc p) d -> p cc d", p=P)
        wv_r = wv.rearrange("(cc p) d -> p cc d", p=P)
        xt_r = xt.rearrange("(cc p) i -> p cc i", p=P)
        yt_r = yt.rearrange("(cc p) i -> p cc i", p=P)
        wo_r = wo.rearrange("(dt p) o -> p dt o", p=P)

        # ---- input DMAs: 3 hw queues (SP/gpsimd/Activation). Class-1
        # (wq/wk dt0 cols, xt, yt ic0 — gates the first score phase) is
        # split evenly across the queue heads; class-2 strictly behind.
        nc.vector.memset(dummy_sb[:], 1.0)
        # class1 gates the first score phase; class2 (yt-ic1, wv) gates the
        # pair-0 fill chains; class3 (wq/wk rest) gates pair-1; wo is last.
        # sync: wq-dt0, xt 0..2, yt-ic0 0..1 | yt-ic1 0..3, wv 0..1 | ...
        nc.sync.dma_start(out=wq_sb[:, :, 0:P], in_=wq_r[:, :, 0:P])
        for cc in range(3):
            nc.sync.dma_start(out=xt_sb[:, cc], in_=xt_r[:, cc])
        for cc in range(2):
            nc.sync.dma_start(out=yt_sb[:, cc, 0:NI], in_=yt_r[:, cc, 0:NI])
        for cc in range(4):
            nc.sync.dma_start(out=yt_sb[:, cc, NI:L], in_=yt_r[:, cc, NI:L])
        nc.sync.dma_start(out=wq_sb[:, :, P:DL], in_=wq_r[:, :, P:DL])
        for cc in range(2):
            nc.sync.dma_start(out=wv_sb[:, cc], in_=wv_r[:, cc])
        for dt in range(2):
            nc.sync.dma_start(out=wo_sb[:, dt], in_=wo_r[:, dt])
        # gpsimd: wk-dt0, xt 3..5, yt-ic0 2..3 | yt-ic1 4..7, wv 2..3 | ...
        nc.gpsimd.dma_start(out=wk_sb[:, :, 0:P], in_=wk_r[:, :, 0:P])
        for cc in range(3, 6):
            nc.gpsimd.dma_start(out=xt_sb[:, cc], in_=xt_r[:, cc])
        for cc in range(2, 4):
            nc.gpsimd.dma_start(out=yt_sb[:, cc, 0:NI], in_=yt_r[:, cc, 0:NI])
        for cc in range(4, 8):
            nc.gpsimd.dma_start(out=yt_sb[:, cc, NI:L], in_=yt_r[:, cc, NI:L])
        nc.gpsimd.dma_start(out=wk_sb[:, :, P:DL], in_=wk_r[:, :, P:DL])
        for cc in range(2, 4):
            nc.gpsimd.dma_start(out=wv_sb[:, cc], in_=wv_r[:, cc])
        for dt in range(2, 4):
            nc.gpsimd.dma_start(out=wo_sb[:, dt], in_=wo_r[:, dt])
        # scalar: xt 6..7, yt-ic0 4..7 | wv 4..7
        for cc in range(6, 8):
            nc.scalar.dma_start(out=xt_sb[:, cc], in_=xt_r[:, cc])
        for cc in range(4, 8):
            nc.scalar.dma_start(out=yt_sb[:, cc, 0:NI], in_=yt_r[:, cc, 0:NI])
        for cc in range(4, 8):
            nc.scalar.dma_start(out=wv_sb[:, cc], in_=wv_r[:, cc])

        # ones blocks of Vaug only (even h: cols DH:P, odd h: cols 0:DH)
        va_r = va_sb.rearrange("p it (h s) -> p it h s", s=P)
        nc.vector.memset(va_r[:, :, 0::2, DH:P], 1.0)
        nc.gpsimd.memset(va_r[:, :, 1::2, 0:DH], 1.0)
        # f32 identity for the tail partition-move matmuls
        from concourse.masks import make_identity

        id_sb = persist.tile([P, P], F32, tag="ident")
        make_identity(nc, id_sb)

        # ---- HAM warm-up: dummy matmuls while input DMA streams in ----
        ps_dummy = ps_wide.tile([P, 2 * NI], F32, tag="wide")
        for _ in range(N_DUMMY):
            nc.tensor.matmul(
                ps_dummy[:, 0:NI], dummy_sb[:, 0:P], dummy_sb[:], start=True, stop=True
            )

        scale = DH**-0.5

        # ---- chain emitters (each a short burst of independent PE work) ----

        DR = mybir.MatmulPerfMode.DoubleRow

        def proj_chain(w_sb, t_sb, rhs_sb, dt, ic):
            ps = ps_acc.tile([P, NI], F32, tag="acc")
            for cc in range(CC):
                nc.tensor.matmul(
                    ps[:],
                    w_sb[:, cc, dt * P : (dt + 1) * P],
                    rhs_sb[:, cc, ic * NI : (ic + 1) * NI],
                    start=(cc == 0),
                    stop=(cc == CC - 1),
                )
            nc.vector.tensor_copy(t_sb[:, dt, ic * NI : (ic + 1) * NI], ps[:])

        def proj_chain2(w_sb, t_sb, rhs_sb, dt):
            # both ic halves in one emission: consecutive matmuls share the
            # stationary operand (bass dedupes the LDWEIGHTS) and there's a
            # single chain-boundary weight-load exposure instead of two
            ps0 = ps_acc.tile([P, NI], F32, tag="acc")
            ps1 = ps_acc.tile([P, NI], F32, tag="acc")
            for cc in range(CC):
                for ic, ps in ((0, ps0), (1, ps1)):
                    nc.tensor.matmul(
                        ps[:],
                        w_sb[:, cc, dt * P : (dt + 1) * P],
                        rhs_sb[:, cc, ic * NI : (ic + 1) * NI],
                        start=(cc == 0),
                        stop=(cc == CC - 1),
                    )
            nc.vector.tensor_copy(t_sb[:, dt, 0:NI], ps0[:])
            nc.vector.tensor_copy(t_sb[:, dt, NI:L], ps1[:])

        def v_chain(jt):
            ps = ps_acc.tile([P, NI], F32, tag="acc")
            for cc in range(CC):
                nc.tensor.matmul(
                    ps[:],
                    yt_sb[:, cc, jt * P : (jt + 1) * P],
                    wv_sb[:, cc, :],
                    start=(cc == 0),
                    stop=(cc == CC - 1),
                )
            va_blk = va_sb[:, jt].rearrange("p (h s) -> p h s", s=P)
            ps_blk = ps.rearrange("p (h s) -> p h s", s=DH)
            nc.vector.tensor_copy(va_blk[:, 0::2, 0:DH], ps_blk[:, 0::2, :])
            nc.vector.tensor_copy(va_blk[:, 1::2, DH:P], ps_blk[:, 1::2, :])

        # Deferred finishers: the normalize needs a SBUF->SBUF DMA between
        # two DVE ops; emitting the post-DMA ops immediately would stall the
        # in-order DVE stream (and the PSUM-releasing copies queued behind
        # it) for the DMA round-trip. Instead each ctx chain queues its
        # post-DMA ops and the next fill slot flushes them.
        deferred = []

        def flush_deferred():
            while deferred:
                deferred.pop(0)()

        def ctx_chain(h, ptile, ic, fast_norm=False):
            dt, r0 = divmod(h * DH, P)
            ct = ps_acc.tile([P, NI], F32, tag="acc")
            for jt in range(IT):
                nc.tensor.matmul(
                    ct[:],
                    va_sb[:, jt, h * P : (h + 1) * P],
                    ptile[:, jt, ic * NI : (ic + 1) * NI],
                    start=(jt == 0),
                    stop=(jt == IT - 1),
                )
            # custom DVE ops (reciprocal) only work at base partition 0, and
            # ctx/rowsum land on complementary partition halves. Mid-kernel
            # the move goes through a SBUF->SBUF DMA whose round-trip hides
            # under later fill slots; at the tail (fast_norm) a tiny
            # identity matmul does the partition move with ~1us less
            # latency at the cost of a PE slot and a PSUM bank.
            rc = small.tile([P, NI], F32, tag="rc")
            if r0 == 0:
                rs = small.tile([P, NI], F32, tag="rs")
                nc.vector.tensor_copy(rs[DH:P, :], ct[DH:P, :])
                if fast_norm:
                    pr = ps_acc.tile([P, NI], F32, tag="acc")
                    nc.tensor.matmul(
                        pr[0:DH, :], id_sb[DH:P, DH:P], rs[DH:P, :],
                        start=True, stop=True,
                    )

                    def fin():
                        nc.vector.reciprocal_approx_fast(rc[0:DH, :], pr[0:DH, :])
                        nc.vector.tensor_mul(
                            cx_sb[0:DH, dt, ic * NI : (ic + 1) * NI],
                            ct[0:DH, :],
                            rc[0:DH, :],
                        )
                else:
                    nc.gpsimd.dma_start(out=rs[0:DH, :], in_=rs[DH:P, :])

                    def fin():
                        nc.vector.reciprocal_approx_fast(rc[0:DH, :], rs[0:DH, :])
                        nc.vector.tensor_mul(
                            cx_sb[0:DH, dt, ic * NI : (ic + 1) * NI],
                            ct[0:DH, :],
                            rc[0:DH, :],
                        )
            else:
                nc.vector.reciprocal_approx_fast(rc[0:DH, :], ct[0:DH, :])
                if fast_norm:
                    pr = ps_acc.tile([P, NI], F32, tag="acc")
                    nc.tensor.matmul(
                        pr[DH:P, :], id_sb[0:DH, 0:DH], rc[0:DH, :],
                        start=True, stop=True,
                    )
                    # DVE can read only one PSUM operand per instruction
                    rc2 = small.tile([P, NI], F32, tag="rc2")
                    nc.vector.tensor_copy(rc2[DH:P, :], pr[DH:P, :])

                    def fin():
                        nc.vector.tensor_mul(
                            cx_sb[DH:P, dt, ic * NI : (ic + 1) * NI],
                            ct[DH:P, :],
                            rc2[DH:P, :],
                        )
                else:
                    nc.gpsimd.dma_start(out=rc[DH:P, :], in_=rc[0:DH, :])

                    def fin():
                        nc.vector.tensor_mul(
                            cx_sb[DH:P, dt, ic * NI : (ic + 1) * NI],
                            ct[DH:P, :],
                            rc[DH:P, :],
                        )

            deferred.append(fin)

        def po2_chain(it, dts, out_ap):
            # dt-major, oc-interleaved: consecutive matmuls share the
            # stationary cx block so the weight load is deduped; one
            # 256KB DMA for both oc halves
            po0 = ps_acc.tile([P, NI], F32, tag="acc")
            po1 = ps_acc.tile([P, NI], F32, tag="acc")
            for k, dt in enumerate(dts):
                for oc, po in ((0, po0), (1, po1)):
                    nc.tensor.matmul(
                        po[:],
                        cx_sb[:, dt, it * P : (it + 1) * P],
                        wo_sb[:, dt, oc * NI : (oc + 1) * NI],
                        start=(k == 0),
                        stop=(k == len(dts) - 1),
                    )
            o_st = small.tile([P, 2 * NI], BF16, tag="ostw")
            nc.vector.tensor_copy(o_st[:, 0:NI], po0[:])
            nc.vector.tensor_copy(o_st[:, NI:L], po1[:])
            out_r = out_ap.rearrange("(it p) o -> it p o", p=P)
            nc.sync.dma_start(out=out_r[it], in_=o_st[:])

        def po_chain(it, oc, dts, out_ap):
            # out-projection partial over the given d-tiles
            po = ps_acc.tile([P, NI], F32, tag="acc")
            for k, dt in enumerate(dts):
                nc.tensor.matmul(
                    po[:],
                    cx_sb[:, dt, it * P : (it + 1) * P],
                    wo_sb[:, dt, oc * NI : (oc + 1) * NI],
                    start=(k == 0),
                    stop=(k == len(dts) - 1),
                )
            o_st = small.tile([P, NI], BF16, tag="ost")
            nc.vector.tensor_copy(o_st[:], po[:])
            out_r = out_ap.rearrange("(it p) o -> it p o", p=P)
            nc.sync.dma_start(
                out=out_r[it, :, oc * NI : (oc + 1) * NI], in_=o_st[:]
            )

        def po_wide_chain(it, dts, out_ap, copy_eng, dma_eng):
            # both oc halves in one 2-bank psum tile (the ST pool slots are
            # idle once the score phases are done); single FD=1024 bf16
            # drain + one 256KB DMA
            po = ps_wide.tile([P, 2 * NI], F32, tag="wide")
            for oc in range(NIC):
                for k, dt in enumerate(dts):
                    nc.tensor.matmul(
                        po[:, oc * NI : (oc + 1) * NI],
                        cx_sb[:, dt, it * P : (it + 1) * P],
                        wo_sb[:, dt, oc * NI : (oc + 1) * NI],
                        start=(k == 0),
                        stop=(k == len(dts) - 1),
                    )
            o_st = small.tile([P, 2 * NI], BF16, tag="ostw")
            if copy_eng == "scalar":
                nc.scalar.copy(o_st[:], po[:])
            else:
                nc.vector.tensor_copy(o_st[:], po[:])
            out_r = out_ap.rearrange("(it p) o -> it p o", p=P)
            dma_eng(out=out_r[it], in_=o_st[:])

        # ---- ST + exp for a head pair, fill chains between steps ----
        # The two heads' score MMs are interleaved so they run in different
        # PE row groups (partitions 0:64 vs 64:128) concurrently.

        def st_pair(hp, fills):
            dt = hp
            ptiles = []
            for h_off in range(2):
                pt_tile = pt_pool.tile([P, IT, L], BF16, tag="pt")
                ptiles.append(pt_tile)
            fills = list(fills)
            for jt in range(IT):
                sts = [
                    ps_wide.tile([P, 2 * NI], F32, tag="wide", name=f"st{h_off}")
                    for h_off in range(2)
                ]
                # h-outer/ic-inner: bass dedupes the LDWEIGHTS across the
                # two ic matmuls of a head, and h1's weight load pulls
                # ahead into its own row group during h0's matmuls, so
                # h1-ic0 runs concurrently with h0-ic1 (3 slots per step).
                # Alternating h0/h1 would force a weight reload per MM
                # that serializes against the in-flight MM's row group.
                for h_off in range(2):
                    r0 = DH * h_off
                    for ic in range(NIC):
                        nc.tensor.matmul(
                            sts[h_off][:, ic * NI : (ic + 1) * NI],
                            kt_sb[r0 : r0 + DH, dt, jt * P : (jt + 1) * P],
                            qt_sb[r0 : r0 + DH, dt, ic * NI : (ic + 1) * NI],
                            start=True,
                            stop=True,
                        )
                for h_off in range(2):
                    nc.scalar.activation(
                        ptiles[h_off][:, jt, :],
                        sts[h_off][:],
                        mybir.ActivationFunctionType.Exp,
                        scale=scale,
                    )
                if jt < len(fills):
                    pending = list(deferred)
                    deferred.clear()
                    for f in fills[jt]:
                        f()
                    for f in pending:
                        f()
            return ptiles

        # ---- schedule ----
        mk = lambda f, *a: (lambda: f(*a))

        # dt0 projections gate the first score phase; only these run before
        # pair 0 (everything else becomes fill work). K's ic1 half stays a
        # fill: it needs yt-ic1 (class-2 DMA) and is only read by ST jt4.
        proj_chain2(wq_sb, qt_sb, xt_sb, 0)
        proj_chain(wk_sb, kt_sb, yt_sb, 0, 0)

        # pair 0: fill with K-dt0-ic1, V chains and QT1/KT1
        pt0 = st_pair(
            0,
            [
                [],
                [mk(proj_chain, wk_sb, kt_sb, yt_sb, 0, 1)],
                [mk(proj_chain2, wq_sb, qt_sb, xt_sb, 1)],
                [],
                [mk(proj_chain, wk_sb, kt_sb, yt_sb, 1, 0)],
                [mk(v_chain, 0)],
                [mk(v_chain, 1)],
                [mk(v_chain, 2)],
            ],
        )
        # KT1-ic1 covers pair-0's exp drain (needed by pair-1 ST jt4)
        proj_chain(wk_sb, kt_sb, yt_sb, 1, 1)

        # pair 1: remaining V chains first (ctx h0 reads all of va), then
        # ctx of heads 0/1 and QT2
        pt1 = st_pair(
            1,
            [
                [mk(v_chain, 3)],
                [mk(v_chain, 4)],
                [mk(v_chain, 5)],
                [mk(v_chain, 6)],
                [mk(v_chain, 7)],
                [mk(ctx_chain, 0, pt0[0], 0)],
                [mk(ctx_chain, 0, pt0[0], 1)],
                [mk(ctx_chain, 1, pt0[1], 0)],
            ],
        )
        ctx_chain(1, pt0[1], 1)
        flush_deferred()
        proj_chain2(wq_sb, qt_sb, xt_sb, 2)
        proj_chain(wk_sb, kt_sb, yt_sb, 2, 0)

        # pair 2: KT2-ic1 first (needed by ST jt4), ctx of heads 2/3, QT3/KT3
        pt2 = st_pair(
            2,
            [
                [mk(proj_chain, wk_sb, kt_sb, yt_sb, 2, 1)],
                [mk(ctx_chain, 2, pt1[0], 0)],
                [mk(ctx_chain, 2, pt1[0], 1)],
                [mk(ctx_chain, 3, pt1[1], 0)],
                [mk(ctx_chain, 3, pt1[1], 1)],
                [mk(proj_chain2, wq_sb, qt_sb, xt_sb, 3)],
                [mk(proj_chain, wk_sb, kt_sb, yt_sb, 3, 0)],
            ],
        )
        proj_chain(wk_sb, kt_sb, yt_sb, 3, 1)

        # pair 3: ctx of heads 4/5, then out-proj partial A over dt 0..2
        # (heads 0..5). poA[8:16] stays after the st loop to cover the
        # final exp drain that the tail ctx chains must wait for.
        poA = [
            mk(po2_chain, it, (0, 1, 2), out_a) for it in range(IT)
        ]
        pt3 = st_pair(
            3,
            [
                [mk(ctx_chain, 4, pt2[0], 0)],
                [mk(ctx_chain, 4, pt2[0], 1)],
                [mk(ctx_chain, 5, pt2[1], 0)],
                [mk(ctx_chain, 5, pt2[1], 1)],
                poA[0:1],
                poA[1:2],
                poA[2:4],
                poA[4:6],
            ],
        )
        for f in poA[6:8]:
            f()
        flush_deferred()

        # tail: the three ctx chains whose normalize round-trips can hide
        # under later work come first (60/70/71), out blocks it0/1 start as
        # soon as fin60/fin70 flush, ctx61 and the rest pipeline behind.
        # dt3 out-projection uses 2-bank-wide chains with drains
        # alternating scalar/vector and output DMAs split across 2 queues.
        def owide(it):
            po_wide_chain(
                it, (3,), out_c,
                "scalar" if it % 2 == 0 else "vector",
                nc.sync.dma_start if it % 2 == 0 else nc.gpsimd.dma_start,
            )

        ctx_chain(6, pt3[0], 0)          # queues fin60
        pending = list(deferred)
        deferred.clear()
        ctx_chain(7, pt3[1], 0)          # queues fin70
        for f in pending:                # fin60
            f()
        pending = list(deferred)
        deferred.clear()
        ctx_chain(7, pt3[1], 1, fast_norm=True)   # queues fin71
        for f in pending:                # fin70
            f()
        owide(0)                         # needs fin60/fin70 only
        owide(1)
        pending = list(deferred)
        deferred.clear()
        ctx_chain(6, pt3[0], 1, fast_norm=True)   # queues fin61
        for f in pending:                # fin71
            f()
        owide(2)
        owide(3)
        flush_deferred()                 # fin61
        for it in range(IT // 2, IT):
            owide(it)


_NC_CACHE = None


def _get_nc():
    global _NC_CACHE
    if _NC_CACHE is None:
        _NC_CACHE = _build_kernel()
    return _NC_CACHE


def kernel(x, y, Wq, Wk, Wv, Wo, _trace=False):
    bf = ml_dtypes.bfloat16
    f8 = ml_dtypes.float8_e4m3
    x = np.asarray(x, np.float32)
    y = np.asarray(y, np.float32)
    xtb = [np.ascontiguousarray(np.asarray(x[b]).T).astype(bf) for b in range(B)]
    ytb = [np.ascontiguousarray(np.asarray(y[b]).T).astype(bf) for b in range(B)]
    wqs = [np.ascontiguousarray(np.asarray(Wq)[:, t * DL : (t + 1) * DL]).astype(bf) for t in range(TP)]
    wks = [np.ascontiguousarray(np.asarray(Wk)[:, t * DL : (t + 1) * DL]).astype(bf) for t in range(TP)]
    wvs = [np.ascontiguousarray(np.asarray(Wv)[:, t * DL : (t + 1) * DL]).astype(bf) for t in range(TP)]
    wos = [np.ascontiguousarray(np.asarray(Wo)[t * DL : (t + 1) * DL, :]).astype(bf) for t in range(TP)]

    in_maps = []
    for b in range(B):
        for t in range(TP):
            in_maps.append(
                {
                    "xt": xtb[b],
                    "yt": ytb[b],
                    "wq": wqs[t],
                    "wk": wks[t],
                    "wv": wvs[t],
                    "wo": wos[t],
                }
            )

    nc = _get_nc()
    res = run_bass_kernel_spmd(
        nc, in_maps, core_ids=list(range(N_CORES)), trace=_trace
    )
    out = np.empty((B, L, U), np.float32)
    for b in range(B):
        out[b] = (
            np.asarray(res.results[2 * b]["out_a"], np.float32)
            + np.asarray(res.results[2 * b]["out_c"], np.float32)
            + np.asarray(res.results[2 * b + 1]["out_a"], np.float32)
            + np.asarray(res.results[2 * b + 1]["out_c"], np.float32)
        )
    if _trace:
        return out, res
    return out
